# revision 9
# baseline (speedup 1.0000x reference)
"""Multi-head attention block on 8 Trainium2 NeuronCores, data-parallel over
batch, fp8 (e4m3) matmul datapath with DoubleRow perf mode.

Shapes (hardcoded): B=8, S=1024, H=16, HD=64, D=1024. One batch element per
core. Host pre-transposes/casts x and the weights to fp8 (weights scaled by
64 for fp8 range; the 1/64 is folded into the movers), and compacts keys
(unmasked first). Masking is applied by ZEROING the v'/ones rows of masked
(tail) keys, so the exp needs no per-key bias and can run on either engine.

Per-core dataflow v2:
  phase 1: qT/kT (fp8) and v' ([128keys, kc, H, 72] + ones col at 64, tail
           rows zeroed) via DoubleRow fp8 matmuls; movers on ScalarE
           (Identity activation, scale=1/64 or the per-key v-mask).
  phase 2 per dt (2 heads), qh outer: per kt ONE row-tiled matmul PAIR
           computes both heads' scoresT[k,q] concurrently (head A in PE rows
           0-63, head B in rows 64-127, separate PSUM banks); E = exp(s/8)
           computed per (kt,qh) over [128, 2head*512] either on ScalarE
           (Exp) or on VectorE via a Schraudolph int8 bit-trick that emits
           fp8 bits directly; ctx' psum [65, 512] per head accumulates
           v'^T @ E DoubleRow over kt pairs (row 64 = denominator).
           Normalize: DVE reciprocal of the den row, GpSimd
           partition_broadcast, DVE mult+fp8-cast into ctxT.
  phase 3: out proj fp8 DoubleRow; t = x + proj/64 (DVE STT); LayerNorm
           stats via bn_stats/bn_aggr, rstd via batched DVE Newton rsqrt
           (no ScalarE table switch!); final apply on GpSimd.
"""
import sys
import time

sys.path.insert(0, "/opt/trn_rl_repo")

import numpy as np
import ml_dtypes

import concourse.bass as bass
import concourse.bacc as bacc
import concourse.tile as tile
from concourse import mybir
from concourse.bass_utils import run_bass_kernel_spmd

F32 = mybir.dt.float32
FP8 = mybir.dt.float8e4
INT32 = mybir.dt.int32
INT8 = mybir.dt.int8
AF = mybir.ActivationFunctionType
DR = mybir.MatmulPerfMode.DoubleRow
MUL = mybir.AluOpType.mult
ADD = mybir.AluOpType.add
RSH = mybir.AluOpType.logical_shift_right

B, S, H, HD = 8, 1024, 16, 64
D = H * HD
EPS = 1e-6
ST = S // 128    # 8 s-tiles
DT = D // 128    # 8 d-tiles
NH = S // 512    # 2 query halves
WS = 64.0        # weight scale for fp8
IWS = 1.0 / WS
VST = 72         # per-head stride in v' (64 v dims + ones col + pad)
SC_L = 11.5416 * 0.125   # Schraudolph exp: 8*log2(e) * score scale
SC_B = 55.654            # bias: 7*8 - 8*log2(1.0308) (round-to-nearest)
RSQRT_C = 0x5F3759DF     # Schraudolph rsqrt seed constant
RECIP_C = 0x7EF311C3     # Schraudolph reciprocal constant

# exp-engine split: (kt, qh) pairs routed to the DVE Schraudolph path;
# everything else uses ScalarE Exp. Tuned for ACT/DVE load balance.
D_SET = {(2, 0), (2, 1), (4, 0)}


def _emit_body(nc, tc, io, cst, kc):
    (x32, out) = io
    (vmask_sb, qT, kT, vp, ctxT, xT_sb, xcT_sb,
     wq_sb, wk_sb, wv_sb, wo_sb, x_sb) = cst
    SC = kc * 128
    kpairs = kc // 2          # full DoubleRow key-tile pairs for AV
    ktail = kc - 2 * kpairs   # 0 or 1 leftover key tile

    # ---------------- phases 1+2 merged: projections feed heads ------
    # PSUM tags: "scp" [128,2,512]x2 = 4 banks (scores pairs, 2 qh in
    # flight); "ctx" [65,512]x2 = 2 banks (qh0 ctx accumulates inline,
    # qh1 ctx deferred to the dt tail); "prj" [128,512]x2 = 2 banks
    # (short-lived projection chunks, isolated so projections emitted
    # mid-chain can fill PE gaps left by the exp round-trip).
    with (
        tc.tile_pool(name="Ep", bufs=4) as Ep,
        tc.tile_pool(name="nrm", bufs=2) as nrm,
        tc.tile_pool(name="ps", bufs=2, space="PSUM") as ps,
        tc.tile_pool(name="scp", bufs=2, space="PSUM") as scps,
    ):
        def vproj(st):
            # v' natural [keys, dcols], scattered to head slots; the
            # per-key scale (1/64, or 0 for masked tail keys) applies
            # the mask.
            for et in range(2):
                vps = ps.tile([128, 512], F32, tag="prj", name="vps")
                for kp in range(0, DT, 2):
                    nc.tensor.matmul(
                        vps,
                        xcT_sb[:, kp:kp + 2, st * 128:(st + 1) * 128],
                        wv_sb[:, kp:kp + 2, et * 512:(et + 1) * 512],
                        start=(kp == 0), stop=(kp == DT - 2),
                        perf_mode=DR)
                nc.scalar.activation(
                    vp[:, st, et * 8:(et + 1) * 8, 0:64],
                    vps.rearrange("p (h j) -> p h j", j=64),
                    AF.Identity, scale=vmask_sb[:, st:st + 1])

        k_tiles = [(i * 512, min(512, SC - i * 512))
                   for i in range((SC + 511) // 512)]
        q_tiles = [(i * 512, 512) for i in range(NH)]

        def project(wsb_t, dstT, rhsT, mg, ntiles):
            for n0, nsz in ntiles:
                qps = ps.tile([128, 512], F32, tag="prj", name="qps")
                for kp in range(0, DT, 2):
                    nc.tensor.matmul(
                        qps[:, 0:nsz],
                        wsb_t[:, kp:kp + 2, mg * 128:(mg + 1) * 128],
                        rhsT[:, kp:kp + 2, n0:n0 + nsz],
                        start=(kp == 0), stop=(kp == DT - 2),
                        perf_mode=DR)
                nc.scalar.activation(
                    dstT[:, mg, n0:n0 + nsz], qps[:, 0:nsz],
                    AF.Identity, scale=IWS)

        def drain(c, hi, dt, q0):
            # normalize: Schraudolph recip of den row (DVE), partition
            # broadcast (GpSimd), multiply + fp8 cast into ctxT (DVE)
            base = hi * 64
            r0 = nrm.tile([1, 512], F32, tag=f"r0{hi}", name="r0")
            nc.vector.tensor_scalar(
                r0.bitcast(INT32), c[64:65, :].bitcast(INT32),
                -1, RECIP_C, op0=MUL, op1=ADD)
            rbc = nrm.tile([64, 512], F32, tag=f"rbc{hi}", name="rbc")
            nc.gpsimd.partition_broadcast(rbc, r0)
            nc.vector.tensor_mul(
                ctxT[base:base + 64, dt, q0:q0 + 512],
                c[0:64, :], rbc)

        def av_pair(cps2, qh, kt, hA, hB, e_pair):
            # DoubleRow AV over the key-tile pair ending at odd kt, or
            # the stride-0 pad trick for an odd-kc tail at even kt.
            last_pair = (kt % 2 == 1 and ktail == 0 and kt == kc - 1)
            if kt % 2 == 1:
                for hi, h in ((0, hA), (1, hB)):
                    nc.tensor.matmul(
                        cps2[hi],
                        vp[:, kt - 1:kt + 1, h, 0:65],
                        e_pair[:, 0:2, hi, :],
                        start=(kt == 1), stop=last_pair,
                        perf_mode=DR)
            else:
                for hi, h in ((0, hA), (1, hB)):
                    e0 = e_pair[:, 0, hi, :]
                    e00 = bass.AP(
                        tensor=e0.tensor, offset=e0.offset,
                        ap=[list(e0.ap[0]), [0, 2]]
                        + [list(a) for a in e0.ap[1:]])
                    nc.tensor.matmul(
                        cps2[hi],
                        vp[:, kt:kt + 2, h, 0:65],
                        e00,
                        start=(kc == 1), stop=True,
                        perf_mode=DR)

        def chain(dt):
            """Attention for heads (2dt, 2dt+1). Scores+exp for both
            query halves per kt (the two exps land on different
            engines); qh0's AV accumulates inline, qh1's AV runs at the
            dt tail from the persisted E tiles."""
            hA, hB = 2 * dt, 2 * dt + 1
            cps0 = [ps.tile([65, 512], F32, tag="ctx", name="cps")
                    for _ in range(2)]
            e_pairs = [[], []]  # per qh: list of E tiles by kt-pair
            for kt in range(kc):
                if kt % 2 == 0:
                    for qh in range(NH):
                        e_pairs[qh].append(
                            Ep.tile([128, 2, 2, 512], FP8, tag=f"E{qh}",
                                    name="e_pair"))
                for qh in range(NH):
                    q0 = qh * 512
                    scp = scps.tile([128, 2, 512], F32, tag="scp",
                                    name="scp")
                    # row-tiled concurrent pair: head A rows 0-63,
                    # head B rows 64-127 of the PE array
                    nc.tensor.matmul(
                        scp[:, 0, :],
                        kT[0:64, dt, kt * 128:(kt + 1) * 128],
                        qT[0:64, dt, q0:q0 + 512],
                        start=True, stop=True)
                    nc.tensor.matmul(
                        scp[:, 1, :],
                        kT[64:128, dt, kt * 128:(kt + 1) * 128],
                        qT[64:128, dt, q0:q0 + 512],
                        start=True, stop=True)
                    if qh == 1 and kt < 3:
                        nc.vector.tensor_scalar(
                            e_pairs[qh][-1][:, kt % 2, :, :].bitcast(INT8),
                            scp, SC_L, SC_B, op0=MUL, op1=ADD)
                    else:
                        nc.scalar.activation(
                            e_pairs[qh][-1][:, kt % 2, :, :], scp, AF.Exp,
                            scale=0.125)
                if kt % 2 == 1 or kt == kc - 1:
                    av_pair(cps0, 0, kt, hA, hB, e_pairs[0][kt // 2])
                # mid-chain projection filler for the next dt
                if kt == 1 and dt + 1 < DT:
                    project(wq_sb, qT, xT_sb, dt + 1, q_tiles)
                if kt == 3 and dt + 1 < DT:
                    project(wk_sb, kT, xcT_sb, dt + 1, k_tiles)
            drain(cps0[0], 0, dt, 0)
            drain(cps0[1], 1, dt, 0)
            # qh1: AV from persisted E tiles, then drain
            cps1 = [ps.tile([65, 512], F32, tag="ctx", name="cps")
                    for _ in range(2)]
            for kt in range(kc):
                if kt % 2 == 1 or kt == kc - 1:
                    av_pair(cps1, 1, kt, hA, hB, e_pairs[1][kt // 2])
            drain(cps1[0], 0, dt, 512)
            drain(cps1[1], 1, dt, 512)

        for st in range(kc):
            vproj(st)
        project(wq_sb, qT, xT_sb, 0, q_tiles)
        project(wk_sb, kT, xcT_sb, 0, k_tiles)
        for dt in range(DT):
            chain(dt)

    # ---------------- phase 3: out proj + LayerNorm ----------------
    with (
        tc.tile_pool(name="epi", bufs=3) as epi,
        tc.tile_pool(name="tke", bufs=1) as tke,
        tc.tile_pool(name="pjps", bufs=2, space="PSUM") as pjps,
    ):
        t_tiles = []
        mvall = tke.tile([128, ST, 2], F32, tag="mva", name="mva")
        for qt in range(ST):
            x_t = x_sb[:, qt, :]
            t = tke.tile([128, D], F32, tag=f"tq{qt}", name="t")
            t_tiles.append(t)
            for et in range(2):
                pps = pjps.tile([128, 512], F32, tag=f"pj{et}", name="pps")
                for dp in range(0, DT, 2):
                    nc.tensor.matmul(
                        pps,
                        ctxT[:, dp:dp + 2, qt * 128:(qt + 1) * 128],
                        wo_sb[:, dp:dp + 2, et * 512:(et + 1) * 512],
                        start=(dp == 0), stop=(dp == DT - 2),
                        perf_mode=DR)
                nc.vector.scalar_tensor_tensor(
                    out=t[:, et * 512:(et + 1) * 512],
                    in0=pps, scalar=IWS,
                    in1=x_t[:, et * 512:(et + 1) * 512],
                    op0=MUL, op1=ADD)
            stats = epi.tile([128, 2, nc.vector.BN_STATS_DIM], F32,
                             tag="stats", name="stats")
            tg = t.rearrange("p (g d) -> p g d", g=2)
            for g in range(2):
                nc.vector.bn_stats(stats[:, g, :], tg[:, g, :])
            nc.vector.bn_aggr(mvall[:, qt, :], stats)

        # batched Newton rsqrt on DVE: rstd = 1/sqrt(var + eps)
        vv = tke.tile([128, ST], F32, tag="nwv", name="nwv")
        nc.vector.tensor_scalar(vv, mvall[:, :, 1], EPS, None, op0=ADD)
        y = tke.tile([128, ST], F32, tag="nwy", name="nwy")
        nc.vector.tensor_scalar(y.bitcast(INT32), vv.bitcast(INT32),
                                1, None, op0=RSH)
        nc.vector.tensor_scalar(y.bitcast(INT32), y.bitcast(INT32),
                                -1, RSQRT_C, op0=MUL, op1=ADD)
        t1 = tke.tile([128, ST], F32, tag="nwt", name="nwt")
        for _ in range(2):  # two Newton iterations
            nc.vector.tensor_mul(t1, vv, y)
            nc.vector.tensor_mul(t1, t1, y)
            nc.vector.tensor_scalar(t1, t1, -0.5, 1.5, op0=MUL, op1=ADD)
            nc.vector.tensor_mul(y, y, t1)
        nmr = tke.tile([128, ST], F32, tag="nwm", name="nwm")
        nc.vector.scalar_tensor_tensor(
            out=nmr, in0=mvall[:, :, 0], scalar=-1.0, in1=y,
            op0=MUL, op1=MUL)

        for qt in range(ST):
            o_t = epi.tile([128, D], F32, tag="ot", name="o_t")
            nc.gpsimd.tensor_scalar(
                o_t, t_tiles[qt], y[:, qt:qt + 1], nmr[:, qt:qt + 1],
                op0=MUL, op1=ADD)
            nc.sync.dma_start(
                out=out[qt * 128:(qt + 1) * 128, :], in_=o_t)


def build_bass(reps=1, kc=8):
    nc = bacc.Bacc("TRN2", target_bir_lowering=False, debug=False)

    SC = kc * 128
    x32 = nc.dram_tensor("x32", [S, D], F32, kind="ExternalInput").ap()
    xT8 = nc.dram_tensor("xT8", [128, DT, S], FP8, kind="ExternalInput").ap()
    xcT8 = nc.dram_tensor("xcT8", [128, DT, SC], FP8,
                          kind="ExternalInput").ap()
    wq8 = nc.dram_tensor("wq8", [128, DT, D], FP8, kind="ExternalInput").ap()
    wk8 = nc.dram_tensor("wk8", [128, DT, D], FP8, kind="ExternalInput").ap()
    wv8 = nc.dram_tensor("wv8", [128, DT, D], FP8, kind="ExternalInput").ap()
    wo8 = nc.dram_tensor("wo8", [128, DT, D], FP8, kind="ExternalInput").ap()
    vmask = nc.dram_tensor("vmask", [128, kc], F32, kind="ExternalInput").ap()
    vones = nc.dram_tensor("vones", [128, kc, H], FP8,
                           kind="ExternalInput").ap()
    out = nc.dram_tensor("out", [S, D], F32, kind="ExternalOutput").ap()
    io = (x32, out)

    with tile.TileContext(nc) as tc:
        with tc.tile_pool(name="const", bufs=1) as const:
            vmask_sb = const.tile([128, kc], F32, name="vmask_sb")
            nc.sync.dma_start(out=vmask_sb, in_=vmask)
            # invocation-constant inputs: loaded once, resident in SBUF
            xT_sb = const.tile([128, DT, S], FP8, name="xT_sb")
            nc.sync.dma_start(out=xT_sb, in_=xT8)
            xcT_sb = const.tile([128, DT, SC], FP8, name="xcT_sb")
            nc.sync.dma_start(out=xcT_sb, in_=xcT8)
            wq_sb = const.tile([128, DT, D], FP8, name="wq_sb")
            nc.sync.dma_start(out=wq_sb, in_=wq8)
            wk_sb = const.tile([128, DT, D], FP8, name="wk_sb")
            nc.sync.dma_start(out=wk_sb, in_=wk8)
            wv_sb = const.tile([128, DT, D], FP8, name="wv_sb")
            nc.sync.dma_start(out=wv_sb, in_=wv8)
            wo_sb = const.tile([128, DT, D], FP8, name="wo_sb")
            nc.sync.dma_start(out=wo_sb, in_=wo8)
            x_sb = const.tile([128, ST, D], F32, name="x_sb")
            nc.sync.dma_start(out=x_sb,
                              in_=x32.rearrange("(t p) d -> p t d", p=128))
            qT = const.tile([128, DT, S], FP8, name="qT")
            kT = const.tile([128, DT, SC], FP8, name="kT")
            vp = const.tile([128, kc + 1, H, VST], FP8, name="vp")
            ctxT = const.tile([128, DT, S], FP8, name="ctxT")
            # ones column of v' (col 64 of each head slot): 1.0 for live
            # keys, 0.0 for masked tail keys (this applies the mask)
            nc.sync.dma_start(out=vp[:, 0:kc, :, 64], in_=vones)
            # zeroed pad key-plane: lets the odd-kc AV tail run DoubleRow
            # with a garbage E plane
            nc.vector.memset(vp[:, kc, :, :], 0.0)
            cst = (vmask_sb, qT, kT, vp, ctxT, xT_sb, xcT_sb,
                   wq_sb, wk_sb, wv_sb, wo_sb, x_sb)
            for _ in range(reps):
                _emit_body(nc, tc, io, cst, kc)

    nc.compile()
    return nc


_NC_CACHE = {}


def _get_nc(reps=1, kc=8):
    if (reps, kc) not in _NC_CACHE:
        _NC_CACHE[(reps, kc)] = build_bass(reps, kc)
    return _NC_CACHE[(reps, kc)]


def _pack_w(w):
    # [D, D] -> [128, DT, D] fp8 with w8[p, t, n] = w[t*128+p, n] * WS
    return np.ascontiguousarray(
        (np.asarray(w, np.float32) * WS).reshape(DT, 128, D)
        .transpose(1, 0, 2)).astype(ml_dtypes.float8_e4m3)


def _pack_xT(x):
    # [S', D] -> [128, DT, S'] fp8 with xT8[p, t, s] = x[s, t*128+p]
    return np.ascontiguousarray(
        np.asarray(x, np.float32).T.reshape(DT, 128, -1)
        .transpose(1, 0, 2)).astype(ml_dtypes.float8_e4m3)


def make_in_maps(x, mask, wq, bq, wk, bk, wv, bv, wo, bo, gamma, beta):
    for b in (bq, bk, bv, bo):
        assert not np.any(np.asarray(b)), "nonzero bias unsupported"
    x = np.asarray(x, dtype=np.float32)
    mask = np.asarray(mask)
    n_un_all = (mask == 0).sum(axis=1)
    kc = min(max((int(n_un_all.max()) + 127) // 128, 2), ST)
    SC = kc * 128
    idxs = [np.argsort(mask[c], kind="stable")[:SC] for c in range(B)]
    key_idx = np.arange(SC).reshape(kc, 128).T  # [128, kc] global key index
    common = {
        "wq8": _pack_w(wq), "wk8": _pack_w(wk),
        "wv8": _pack_w(wv), "wo8": _pack_w(wo),
    }
    maps = []
    for c in range(B):
        xc = x[c][idxs[c]]
        live = (key_idx < int(n_un_all[c]))  # [128, kc]
        maps.append(dict(
            common,
            x32=np.ascontiguousarray(x[c]),
            xT8=_pack_xT(x[c]),
            xcT8=_pack_xT(xc),
            vmask=np.ascontiguousarray(live.astype(np.float32) * IWS),
            vones=np.ascontiguousarray(
                np.broadcast_to(live[:, :, None], (128, kc, H))
                .astype(ml_dtypes.float8_e4m3))))
    return maps, kc


def kernel(x, mask, wq, bq, wk, bk, wv, bv, wo, bo, gamma, beta):
    in_maps, kc = make_in_maps(x, mask, wq, bq, wk, bk, wv, bv, wo, bo,
                               gamma, beta)
    nc = _get_nc(1, kc)
    last_err = None
    for _ in range(3):
        try:
            res = run_bass_kernel_spmd(nc, in_maps, core_ids=list(range(B)))
            outv = np.stack([res.results[c]["out"] for c in range(B)], axis=0)
            gamma = np.asarray(gamma, np.float32)
            beta = np.asarray(beta, np.float32)
            if not (np.all(gamma == 1.0) and np.all(beta == 0.0)):
                outv = outv * gamma + beta
            return outv
        except Exception as e:  # transient NRT device errors: retry
            last_err = e
            time.sleep(5)
    raise last_err


# revision 10
# speedup vs baseline: 1.0407x; 1.0407x over previous
"""Multi-head attention block on 8 Trainium2 NeuronCores, data-parallel over
batch, fp8 (e4m3) matmul datapath with DoubleRow perf mode.

Shapes (hardcoded): B=8, S=1024, H=16, HD=64, D=1024. One batch element per
core. Host pre-transposes/casts x and the weights to fp8 (weights scaled by
64 for fp8 range; the 1/64 is folded into the movers), and compacts keys
(unmasked first). Masking is applied by ZEROING the v'/ones rows of masked
(tail) keys, so the exp needs no per-key bias and can run on either engine.

Per-core dataflow v2:
  phase 1: qT/kT (fp8) and v' ([128keys, kc, H, 72] + ones col at 64, tail
           rows zeroed) via DoubleRow fp8 matmuls; movers on ScalarE
           (Identity activation, scale=1/64 or the per-key v-mask).
  phase 2 per dt (2 heads), qh outer: per kt ONE row-tiled matmul PAIR
           computes both heads' scoresT[k,q] concurrently (head A in PE rows
           0-63, head B in rows 64-127, separate PSUM banks); E = exp(s/8)
           computed per (kt,qh) over [128, 2head*512] either on ScalarE
           (Exp) or on VectorE via a Schraudolph int8 bit-trick that emits
           fp8 bits directly; ctx' psum [65, 512] per head accumulates
           v'^T @ E DoubleRow over kt pairs (row 64 = denominator).
           Normalize: DVE reciprocal of the den row, GpSimd
           partition_broadcast, DVE mult+fp8-cast into ctxT.
  phase 3: out proj fp8 DoubleRow; t = x + proj/64 (DVE STT); LayerNorm
           stats via bn_stats/bn_aggr, rstd via batched DVE Newton rsqrt
           (no ScalarE table switch!); final apply on GpSimd.
"""
import sys
import time

sys.path.insert(0, "/opt/trn_rl_repo")

import numpy as np
import ml_dtypes

import concourse.bass as bass
import concourse.bacc as bacc
import concourse.tile as tile
from concourse import mybir
from concourse.bass_utils import run_bass_kernel_spmd

F32 = mybir.dt.float32
FP8 = mybir.dt.float8e4
INT32 = mybir.dt.int32
INT8 = mybir.dt.int8
AF = mybir.ActivationFunctionType
DR = mybir.MatmulPerfMode.DoubleRow
MUL = mybir.AluOpType.mult
ADD = mybir.AluOpType.add
RSH = mybir.AluOpType.logical_shift_right

B, S, H, HD = 8, 1024, 16, 64
D = H * HD
EPS = 1e-6
ST = S // 128    # 8 s-tiles
DT = D // 128    # 8 d-tiles
NH = S // 512    # 2 query halves
WS = 64.0        # weight scale for fp8
IWS = 1.0 / WS
VST = 72         # per-head stride in v' (64 v dims + ones col + pad)
SC_L = 11.5416 * 0.125   # Schraudolph exp: 8*log2(e) * score scale
SC_B = 55.654            # bias: 7*8 - 8*log2(1.0308) (round-to-nearest)
RSQRT_C = 0x5F3759DF     # Schraudolph rsqrt seed constant
RECIP_C = 0x7EF311C3     # Schraudolph reciprocal constant

# exp-engine split: (kt, qh) pairs routed to the DVE Schraudolph path;
# everything else uses ScalarE Exp. Tuned for ACT/DVE load balance.
D_SET = {(2, 0), (2, 1), (4, 0)}


def _emit_body(nc, tc, io, cst, kc):
    (x32, out) = io
    (vmask_sb, qT, kT, vp, ctxT, xT_sb, xcT_sb,
     wq_sb, wk_sb, wv_sb, wo_sb, x_sb) = cst
    SC = kc * 128
    kpairs = kc // 2          # full DoubleRow key-tile pairs for AV
    ktail = kc - 2 * kpairs   # 0 or 1 leftover key tile

    # ---------------- phases 1+2 merged: projections feed heads ------
    # PSUM tags: "scp" [128,2,512]x2 = 4 banks (scores pairs, 2 qh in
    # flight); "ctx" [65,512]x2 = 2 banks (qh0 ctx accumulates inline,
    # qh1 ctx deferred to the dt tail); "prj" [128,512]x2 = 2 banks
    # (short-lived projection chunks, isolated so projections emitted
    # mid-chain can fill PE gaps left by the exp round-trip).
    with (
        tc.tile_pool(name="Ep", bufs=4) as Ep,
        tc.tile_pool(name="nrm", bufs=2) as nrm,
        tc.tile_pool(name="ps", bufs=2, space="PSUM") as ps,
        tc.tile_pool(name="scp", bufs=2, space="PSUM") as scps,
    ):
        def vproj(st):
            # v' natural [keys, dcols], scattered to head slots; the
            # per-key scale (1/64, or 0 for masked tail keys) applies
            # the mask.
            for et in range(2):
                vps = ps.tile([128, 512], F32, tag="prj", name="vps")
                for kp in range(0, DT, 2):
                    nc.tensor.matmul(
                        vps,
                        xcT_sb[:, kp:kp + 2, st * 128:(st + 1) * 128],
                        wv_sb[:, kp:kp + 2, et * 512:(et + 1) * 512],
                        start=(kp == 0), stop=(kp == DT - 2),
                        perf_mode=DR)
                nc.scalar.activation(
                    vp[:, st, et * 8:(et + 1) * 8, 0:64],
                    vps.rearrange("p (h j) -> p h j", j=64),
                    AF.Identity, scale=vmask_sb[:, st:st + 1])

        k_tiles = [(i * 512, min(512, SC - i * 512))
                   for i in range((SC + 511) // 512)]
        q_tiles = [(i * 512, 512) for i in range(NH)]

        def project(wsb_t, dstT, rhsT, mg, ntiles):
            for n0, nsz in ntiles:
                qps = ps.tile([128, 512], F32, tag="prj", name="qps")
                for kp in range(0, DT, 2):
                    nc.tensor.matmul(
                        qps[:, 0:nsz],
                        wsb_t[:, kp:kp + 2, mg * 128:(mg + 1) * 128],
                        rhsT[:, kp:kp + 2, n0:n0 + nsz],
                        start=(kp == 0), stop=(kp == DT - 2),
                        perf_mode=DR)
                nc.scalar.activation(
                    dstT[:, mg, n0:n0 + nsz], qps[:, 0:nsz],
                    AF.Identity, scale=IWS)

        def drain(c, hi, dt, q0):
            # normalize: Schraudolph recip of den row (DVE), partition
            # broadcast (GpSimd), multiply + fp8 cast into ctxT (DVE)
            base = hi * 64
            r0 = nrm.tile([1, 512], F32, tag=f"r0{hi}", name="r0")
            nc.vector.tensor_scalar(
                r0.bitcast(INT32), c[64:65, :].bitcast(INT32),
                -1, RECIP_C, op0=MUL, op1=ADD)
            rbc = nrm.tile([64, 512], F32, tag=f"rbc{hi}", name="rbc")
            nc.gpsimd.partition_broadcast(rbc, r0)
            nc.vector.tensor_mul(
                ctxT[base:base + 64, dt, q0:q0 + 512],
                c[0:64, :], rbc)

        def av_pair(cps2, qh, kt, hA, hB, e_pair):
            # DoubleRow AV over the key-tile pair ending at odd kt, or
            # the stride-0 pad trick for an odd-kc tail at even kt.
            last_pair = (kt % 2 == 1 and ktail == 0 and kt == kc - 1)
            if kt % 2 == 1:
                for hi, h in ((0, hA), (1, hB)):
                    nc.tensor.matmul(
                        cps2[hi],
                        vp[:, kt - 1:kt + 1, h, 0:65],
                        e_pair[:, 0:2, hi, :],
                        start=(kt == 1), stop=last_pair,
                        perf_mode=DR)
            else:
                for hi, h in ((0, hA), (1, hB)):
                    e0 = e_pair[:, 0, hi, :]
                    e00 = bass.AP(
                        tensor=e0.tensor, offset=e0.offset,
                        ap=[list(e0.ap[0]), [0, 2]]
                        + [list(a) for a in e0.ap[1:]])
                    nc.tensor.matmul(
                        cps2[hi],
                        vp[:, kt:kt + 2, h, 0:65],
                        e00,
                        start=(kc == 1), stop=True,
                        perf_mode=DR)

        def dummy_mm(c, rhs_view):
            # HAM keepalive: accumulate exact zeros (vp pad plane,
            # stride-0 pair) into a live ctx psum so the PE array never
            # idles long enough to re-throttle to K=4/8.
            z0 = vp[:, kc, 0, 0:65]
            z00 = bass.AP(
                tensor=z0.tensor, offset=z0.offset,
                ap=[list(z0.ap[0]), [0, 2]] + [list(a) for a in z0.ap[1:]])
            nc.tensor.matmul(c, z00, rhs_view, start=False, stop=False,
                             perf_mode=DR, skip_group_check=True)

        def chain(dt, qh):
            """Attention for heads (2dt, 2dt+1) over query half qh."""
            hA, hB = 2 * dt, 2 * dt + 1
            q0 = qh * 512
            cps = [ps.tile([65, 512], F32, tag="ctx", name="cps")
                   for _ in range(2)]
            dummy_rhs = qT[:, dt, 0:1024].rearrange("p (a b) -> p a b", a=2)
            e_pair = None
            for kt in range(kc):
                scp = scps.tile([128, 2, 512], F32, tag="scp", name="scp")
                # row-tiled concurrent pair: head A rows 0-63, head B
                # rows 64-127 of the PE array
                nc.tensor.matmul(
                    scp[:, 0, :],
                    kT[0:64, dt, kt * 128:(kt + 1) * 128],
                    qT[0:64, dt, q0:q0 + 512],
                    start=True, stop=True)
                nc.tensor.matmul(
                    scp[:, 1, :],
                    kT[64:128, dt, kt * 128:(kt + 1) * 128],
                    qT[64:128, dt, q0:q0 + 512],
                    start=True, stop=True)
                if kt % 2 == 0:
                    e_pair = Ep.tile([128, 2, 2, 512], FP8, tag=f"E{qh}",
                                     name="e_pair")
                if (kt, qh) in D_SET:
                    nc.vector.tensor_scalar(
                        e_pair[:, kt % 2, :, :].bitcast(INT8), scp,
                        SC_L, SC_B, op0=MUL, op1=ADD)
                else:
                    nc.scalar.activation(
                        e_pair[:, kt % 2, :, :], scp, AF.Exp, scale=0.125)
                if kt % 2 == 1 or kt == kc - 1:
                    av_pair(cps, qh, kt, hA, hB, e_pair)
                if 0 < kt < kc - 1:
                    dummy_mm(cps[0], dummy_rhs)
                    dummy_mm(cps[1], dummy_rhs)
            drain(cps[0], 0, dt, q0)
            drain(cps[1], 1, dt, q0)

        for st in range(kc):
            vproj(st)
        project(wq_sb, qT, xT_sb, 0, q_tiles)
        project(wk_sb, kT, xcT_sb, 0, k_tiles)
        for dt in range(DT):
            chain(dt, 0)
            if dt + 1 < DT:
                project(wq_sb, qT, xT_sb, dt + 1, q_tiles)
            chain(dt, 1)
            if dt + 1 < DT:
                project(wk_sb, kT, xcT_sb, dt + 1, k_tiles)

    # ---------------- phase 3: out proj + LayerNorm ----------------
    with (
        tc.tile_pool(name="epi", bufs=3) as epi,
        tc.tile_pool(name="tke", bufs=1) as tke,
        tc.tile_pool(name="pjps", bufs=2, space="PSUM") as pjps,
    ):
        t_tiles = []
        mvall = tke.tile([128, ST, 2], F32, tag="mva", name="mva")
        for qt in range(ST):
            x_t = x_sb[:, qt, :]
            t = tke.tile([128, D], F32, tag=f"tq{qt}", name="t")
            t_tiles.append(t)
            for et in range(2):
                pps = pjps.tile([128, 512], F32, tag=f"pj{et}", name="pps")
                for dp in range(0, DT, 2):
                    nc.tensor.matmul(
                        pps,
                        ctxT[:, dp:dp + 2, qt * 128:(qt + 1) * 128],
                        wo_sb[:, dp:dp + 2, et * 512:(et + 1) * 512],
                        start=(dp == 0), stop=(dp == DT - 2),
                        perf_mode=DR)
                nc.vector.scalar_tensor_tensor(
                    out=t[:, et * 512:(et + 1) * 512],
                    in0=pps, scalar=IWS,
                    in1=x_t[:, et * 512:(et + 1) * 512],
                    op0=MUL, op1=ADD)
            stats = epi.tile([128, 2, nc.vector.BN_STATS_DIM], F32,
                             tag="stats", name="stats")
            tg = t.rearrange("p (g d) -> p g d", g=2)
            for g in range(2):
                nc.vector.bn_stats(stats[:, g, :], tg[:, g, :])
            nc.vector.bn_aggr(mvall[:, qt, :], stats)

        # batched Newton rsqrt on DVE: rstd = 1/sqrt(var + eps)
        vv = tke.tile([128, ST], F32, tag="nwv", name="nwv")
        nc.vector.tensor_scalar(vv, mvall[:, :, 1], EPS, None, op0=ADD)
        y = tke.tile([128, ST], F32, tag="nwy", name="nwy")
        nc.vector.tensor_scalar(y.bitcast(INT32), vv.bitcast(INT32),
                                1, None, op0=RSH)
        nc.vector.tensor_scalar(y.bitcast(INT32), y.bitcast(INT32),
                                -1, RSQRT_C, op0=MUL, op1=ADD)
        t1 = tke.tile([128, ST], F32, tag="nwt", name="nwt")
        for _ in range(2):  # two Newton iterations
            nc.vector.tensor_mul(t1, vv, y)
            nc.vector.tensor_mul(t1, t1, y)
            nc.vector.tensor_scalar(t1, t1, -0.5, 1.5, op0=MUL, op1=ADD)
            nc.vector.tensor_mul(y, y, t1)
        nmr = tke.tile([128, ST], F32, tag="nwm", name="nwm")
        nc.vector.scalar_tensor_tensor(
            out=nmr, in0=mvall[:, :, 0], scalar=-1.0, in1=y,
            op0=MUL, op1=MUL)

        for qt in range(ST):
            o_t = epi.tile([128, D], F32, tag="ot", name="o_t")
            nc.gpsimd.tensor_scalar(
                o_t, t_tiles[qt], y[:, qt:qt + 1], nmr[:, qt:qt + 1],
                op0=MUL, op1=ADD)
            nc.sync.dma_start(
                out=out[qt * 128:(qt + 1) * 128, :], in_=o_t)


def build_bass(reps=1, kc=8):
    nc = bacc.Bacc("TRN2", target_bir_lowering=False, debug=False)

    SC = kc * 128
    x32 = nc.dram_tensor("x32", [S, D], F32, kind="ExternalInput").ap()
    xT8 = nc.dram_tensor("xT8", [128, DT, S], FP8, kind="ExternalInput").ap()
    xcT8 = nc.dram_tensor("xcT8", [128, DT, SC], FP8,
                          kind="ExternalInput").ap()
    wq8 = nc.dram_tensor("wq8", [128, DT, D], FP8, kind="ExternalInput").ap()
    wk8 = nc.dram_tensor("wk8", [128, DT, D], FP8, kind="ExternalInput").ap()
    wv8 = nc.dram_tensor("wv8", [128, DT, D], FP8, kind="ExternalInput").ap()
    wo8 = nc.dram_tensor("wo8", [128, DT, D], FP8, kind="ExternalInput").ap()
    vmask = nc.dram_tensor("vmask", [128, kc], F32, kind="ExternalInput").ap()
    vones = nc.dram_tensor("vones", [128, kc, H], FP8,
                           kind="ExternalInput").ap()
    out = nc.dram_tensor("out", [S, D], F32, kind="ExternalOutput").ap()
    io = (x32, out)

    with tile.TileContext(nc) as tc:
        with tc.tile_pool(name="const", bufs=1) as const:
            vmask_sb = const.tile([128, kc], F32, name="vmask_sb")
            nc.sync.dma_start(out=vmask_sb, in_=vmask)
            # invocation-constant inputs: loaded once, resident in SBUF
            xT_sb = const.tile([128, DT, S], FP8, name="xT_sb")
            nc.sync.dma_start(out=xT_sb, in_=xT8)
            xcT_sb = const.tile([128, DT, SC], FP8, name="xcT_sb")
            nc.sync.dma_start(out=xcT_sb, in_=xcT8)
            wq_sb = const.tile([128, DT, D], FP8, name="wq_sb")
            nc.sync.dma_start(out=wq_sb, in_=wq8)
            wk_sb = const.tile([128, DT, D], FP8, name="wk_sb")
            nc.sync.dma_start(out=wk_sb, in_=wk8)
            wv_sb = const.tile([128, DT, D], FP8, name="wv_sb")
            nc.sync.dma_start(out=wv_sb, in_=wv8)
            wo_sb = const.tile([128, DT, D], FP8, name="wo_sb")
            nc.sync.dma_start(out=wo_sb, in_=wo8)
            x_sb = const.tile([128, ST, D], F32, name="x_sb")
            nc.sync.dma_start(out=x_sb,
                              in_=x32.rearrange("(t p) d -> p t d", p=128))
            qT = const.tile([128, DT, S], FP8, name="qT")
            kT = const.tile([128, DT, SC], FP8, name="kT")
            vp = const.tile([128, kc + 1, H, VST], FP8, name="vp")
            ctxT = const.tile([128, DT, S], FP8, name="ctxT")
            # ones column of v' (col 64 of each head slot): 1.0 for live
            # keys, 0.0 for masked tail keys (this applies the mask)
            nc.sync.dma_start(out=vp[:, 0:kc, :, 64], in_=vones)
            # zeroed pad key-plane: lets the odd-kc AV tail run DoubleRow
            # with a garbage E plane
            nc.vector.memset(vp[:, kc, :, :], 0.0)
            cst = (vmask_sb, qT, kT, vp, ctxT, xT_sb, xcT_sb,
                   wq_sb, wk_sb, wv_sb, wo_sb, x_sb)
            for _ in range(reps):
                _emit_body(nc, tc, io, cst, kc)

    nc.compile()
    return nc


_NC_CACHE = {}


def _get_nc(reps=1, kc=8):
    if (reps, kc) not in _NC_CACHE:
        _NC_CACHE[(reps, kc)] = build_bass(reps, kc)
    return _NC_CACHE[(reps, kc)]


def _pack_w(w):
    # [D, D] -> [128, DT, D] fp8 with w8[p, t, n] = w[t*128+p, n] * WS
    return np.ascontiguousarray(
        (np.asarray(w, np.float32) * WS).reshape(DT, 128, D)
        .transpose(1, 0, 2)).astype(ml_dtypes.float8_e4m3)


def _pack_xT(x):
    # [S', D] -> [128, DT, S'] fp8 with xT8[p, t, s] = x[s, t*128+p]
    return np.ascontiguousarray(
        np.asarray(x, np.float32).T.reshape(DT, 128, -1)
        .transpose(1, 0, 2)).astype(ml_dtypes.float8_e4m3)


def make_in_maps(x, mask, wq, bq, wk, bk, wv, bv, wo, bo, gamma, beta):
    for b in (bq, bk, bv, bo):
        assert not np.any(np.asarray(b)), "nonzero bias unsupported"
    x = np.asarray(x, dtype=np.float32)
    mask = np.asarray(mask)
    n_un_all = (mask == 0).sum(axis=1)
    kc = min(max((int(n_un_all.max()) + 127) // 128, 2), ST)
    SC = kc * 128
    idxs = [np.argsort(mask[c], kind="stable")[:SC] for c in range(B)]
    key_idx = np.arange(SC).reshape(kc, 128).T  # [128, kc] global key index
    common = {
        "wq8": _pack_w(wq), "wk8": _pack_w(wk),
        "wv8": _pack_w(wv), "wo8": _pack_w(wo),
    }
    maps = []
    for c in range(B):
        xc = x[c][idxs[c]]
        live = (key_idx < int(n_un_all[c]))  # [128, kc]
        maps.append(dict(
            common,
            x32=np.ascontiguousarray(x[c]),
            xT8=_pack_xT(x[c]),
            xcT8=_pack_xT(xc),
            vmask=np.ascontiguousarray(live.astype(np.float32) * IWS),
            vones=np.ascontiguousarray(
                np.broadcast_to(live[:, :, None], (128, kc, H))
                .astype(ml_dtypes.float8_e4m3))))
    return maps, kc


def kernel(x, mask, wq, bq, wk, bk, wv, bv, wo, bo, gamma, beta):
    in_maps, kc = make_in_maps(x, mask, wq, bq, wk, bk, wv, bv, wo, bo,
                               gamma, beta)
    nc = _get_nc(1, kc)
    last_err = None
    for _ in range(3):
        try:
            res = run_bass_kernel_spmd(nc, in_maps, core_ids=list(range(B)))
            outv = np.stack([res.results[c]["out"] for c in range(B)], axis=0)
            gamma = np.asarray(gamma, np.float32)
            beta = np.asarray(beta, np.float32)
            if not (np.all(gamma == 1.0) and np.all(beta == 0.0)):
                outv = outv * gamma + beta
            return outv
        except Exception as e:  # transient NRT device errors: retry
            last_err = e
            time.sleep(5)
    raise last_err


# revision 13
# speedup vs baseline: 1.0641x; 1.0224x over previous
"""Multi-head attention block on 8 Trainium2 NeuronCores, data-parallel over
batch, fp8 (e4m3) matmul datapath with DoubleRow perf mode.

Shapes (hardcoded): B=8, S=1024, H=16, HD=64, D=1024. One batch element per
core. Host pre-transposes/casts x and the weights to fp8 (weights scaled by
64 for fp8 range; the 1/64 is folded into the movers), and compacts keys
(unmasked first). Masking is applied by ZEROING the v'/ones rows of masked
(tail) keys, so the exp needs no per-key bias and can run on either engine.

Per-core dataflow v2:
  phase 1: qT/kT (fp8) and v' ([128keys, kc, H, 72] + ones col at 64, tail
           rows zeroed) via DoubleRow fp8 matmuls; movers on ScalarE
           (Identity activation, scale=1/64 or the per-key v-mask).
  phase 2 per dt (2 heads), qh outer: per kt ONE row-tiled matmul PAIR
           computes both heads' scoresT[k,q] concurrently (head A in PE rows
           0-63, head B in rows 64-127, separate PSUM banks); E = exp(s/8)
           computed per (kt,qh) over [128, 2head*512] either on ScalarE
           (Exp) or on VectorE via a Schraudolph int8 bit-trick that emits
           fp8 bits directly; ctx' psum [65, 512] per head accumulates
           v'^T @ E DoubleRow over kt pairs (row 64 = denominator).
           Normalize: DVE reciprocal of the den row, GpSimd
           partition_broadcast, DVE mult+fp8-cast into ctxT.
  phase 3: out proj fp8 DoubleRow; t = x + proj/64 (DVE STT); LayerNorm
           stats via bn_stats/bn_aggr, rstd via batched DVE Newton rsqrt
           (no ScalarE table switch!); final apply on GpSimd.
"""
import sys
import time

sys.path.insert(0, "/opt/trn_rl_repo")

import numpy as np
import ml_dtypes

import concourse.bass as bass
import concourse.bacc as bacc
import concourse.tile as tile
from concourse import mybir
from concourse.bass_utils import run_bass_kernel_spmd

F32 = mybir.dt.float32
FP8 = mybir.dt.float8e4
INT32 = mybir.dt.int32
INT8 = mybir.dt.int8
AF = mybir.ActivationFunctionType
DR = mybir.MatmulPerfMode.DoubleRow
MUL = mybir.AluOpType.mult
ADD = mybir.AluOpType.add
RSH = mybir.AluOpType.logical_shift_right

B, S, H, HD = 8, 1024, 16, 64
D = H * HD
EPS = 1e-6
ST = S // 128    # 8 s-tiles
DT = D // 128    # 8 d-tiles
NH = S // 512    # 2 query halves
WS = 64.0        # weight scale for fp8
IWS = 1.0 / WS
VST = 72         # per-head stride in v' (64 v dims + ones col + pad)
SC_L = 11.5416 * 0.125   # Schraudolph exp: 8*log2(e) * score scale
SC_B = 55.654            # bias: 7*8 - 8*log2(1.0308) (round-to-nearest)
RSQRT_C = 0x5F3759DF     # Schraudolph rsqrt seed constant
RECIP_C = 0x7EF311C3     # Schraudolph reciprocal constant

# exp-engine split: (kt, qh) pairs routed to the DVE Schraudolph path;
# everything else uses ScalarE Exp. Tuned for ACT/DVE load balance.
D_SET = {(2, 0), (2, 1), (4, 0)}


def _emit_body(nc, tc, io, cst, kc):
    (x32, out) = io
    (vmask_sb, qT, kT, vp, ctxT, xT_sb, xcT_sb,
     wq_sb, wk_sb, wv_sb, wo_sb, x_sb) = cst
    SC = kc * 128
    kpairs = kc // 2          # full DoubleRow key-tile pairs for AV
    ktail = kc - 2 * kpairs   # 0 or 1 leftover key tile

    # ---------------- phases 1+2 merged: projections feed heads ------
    # PSUM tags: "scp" [128,2,512]x2 = 4 banks (scores pairs, 2 qh in
    # flight); "ctx" [65,512]x2 = 2 banks (qh0 ctx accumulates inline,
    # qh1 ctx deferred to the dt tail); "prj" [128,512]x2 = 2 banks
    # (short-lived projection chunks, isolated so projections emitted
    # mid-chain can fill PE gaps left by the exp round-trip).
    with (
        tc.tile_pool(name="Ep", bufs=4) as Ep,
        tc.tile_pool(name="nrm", bufs=2) as nrm,
        tc.tile_pool(name="ps", bufs=2, space="PSUM") as ps,
        tc.tile_pool(name="scp", bufs=3, space="PSUM") as scps,
    ):
        def vproj(st):
            # v' natural [keys, dcols], scattered to head slots; the
            # per-key scale (1/64, or 0 for masked tail keys) applies
            # the mask. Both 512-col halves go into one scp-ring tile,
            # drained by a single merged mover.
            vps = scps.tile([128, 2, 512], F32, tag="scp", name="vps")
            for et in range(2):
                for kp in range(0, DT, 2):
                    nc.tensor.matmul(
                        vps[:, et, :],
                        xcT_sb[:, kp:kp + 2, st * 128:(st + 1) * 128],
                        wv_sb[:, kp:kp + 2, et * 512:(et + 1) * 512],
                        start=(kp == 0), stop=(kp == DT - 2),
                        perf_mode=DR)
            nc.scalar.activation(
                vp[:, st, :, 0:64],
                vps.rearrange("p e (h j) -> p (e h) j", j=64),
                AF.Identity, scale=vmask_sb[:, st:st + 1])

        k_tiles = [(i * 512, min(512, SC - i * 512))
                   for i in range((SC + 511) // 512)]
        q_tiles = [(i * 512, 512) for i in range(NH)]

        def project(wsb_t, dstT, rhsT, mg, ntiles):
            qps = scps.tile([128, 2, 512], F32, tag="scp", name="qps")
            tot = 0
            for ci, (n0, nsz) in enumerate(ntiles):
                for kp in range(0, DT, 2):
                    nc.tensor.matmul(
                        qps[:, ci, 0:nsz],
                        wsb_t[:, kp:kp + 2, mg * 128:(mg + 1) * 128],
                        rhsT[:, kp:kp + 2, n0:n0 + nsz],
                        start=(kp == 0), stop=(kp == DT - 2),
                        perf_mode=DR)
                tot += nsz
            nc.scalar.activation(
                dstT[:, mg, 0:tot],
                qps.rearrange("p a b -> p (a b)")[:, 0:tot],
                AF.Identity, scale=IWS)

        def drain(c, hi, dt, q0):
            # normalize: Schraudolph recip of den row (DVE), partition
            # broadcast (GpSimd), multiply + fp8 cast into ctxT (DVE)
            base = hi * 64
            r0 = nrm.tile([1, 512], F32, tag=f"r0{hi}", name="r0")
            nc.vector.tensor_scalar(
                r0.bitcast(INT32), c[64:65, :].bitcast(INT32),
                -1, RECIP_C, op0=MUL, op1=ADD)
            rbc = nrm.tile([64, 512], F32, tag=f"rbc{hi}", name="rbc")
            nc.gpsimd.partition_broadcast(rbc, r0)
            nc.vector.tensor_mul(
                ctxT[base:base + 64, dt, q0:q0 + 512],
                c[0:64, :], rbc)

        def av_pair(cps2, qh, kt, hA, hB, e_pair):
            # DoubleRow AV over the key-tile pair ending at odd kt, or
            # the stride-0 pad trick for an odd-kc tail at even kt.
            last_pair = (kt % 2 == 1 and ktail == 0 and kt == kc - 1)
            if kt % 2 == 1:
                for hi, h in ((0, hA), (1, hB)):
                    nc.tensor.matmul(
                        cps2[hi],
                        vp[:, kt - 1:kt + 1, h, 0:65],
                        e_pair[:, 0:2, hi, :],
                        start=(kt == 1), stop=last_pair,
                        perf_mode=DR)
            else:
                for hi, h in ((0, hA), (1, hB)):
                    e0 = e_pair[:, 0, hi, :]
                    e00 = bass.AP(
                        tensor=e0.tensor, offset=e0.offset,
                        ap=[list(e0.ap[0]), [0, 2]]
                        + [list(a) for a in e0.ap[1:]])
                    nc.tensor.matmul(
                        cps2[hi],
                        vp[:, kt:kt + 2, h, 0:65],
                        e00,
                        start=(kc == 1), stop=True,
                        perf_mode=DR)

        def dummy_mm(c, rhs_view):
            # HAM keepalive: accumulate exact zeros (vp pad plane,
            # stride-0 pair) into a live ctx psum so the PE array never
            # idles long enough to re-throttle to K=4/8.
            z0 = vp[:, kc, 0, 0:65]
            z00 = bass.AP(
                tensor=z0.tensor, offset=z0.offset,
                ap=[list(z0.ap[0]), [0, 2]] + [list(a) for a in z0.ap[1:]])
            nc.tensor.matmul(c, z00, rhs_view, start=False, stop=False,
                             perf_mode=DR, skip_group_check=True)

        def chain(dt, qh):
            """Attention for heads (2dt, 2dt+1) over query half qh."""
            hA, hB = 2 * dt, 2 * dt + 1
            q0 = qh * 512
            cps = [ps.tile([65, 512], F32, tag="ctx", name="cps")
                   for _ in range(2)]
            dummy_rhs = qT[:, dt, 0:1024].rearrange("p (a b) -> p a b", a=2)
            e_pair = None
            for kt in range(kc):
                scp = scps.tile([128, 2, 512], F32, tag="scp", name="scp")
                # row-tiled concurrent pair: head A rows 0-63, head B
                # rows 64-127 of the PE array
                nc.tensor.matmul(
                    scp[:, 0, :],
                    kT[0:64, dt, kt * 128:(kt + 1) * 128],
                    qT[0:64, dt, q0:q0 + 512],
                    start=True, stop=True)
                nc.tensor.matmul(
                    scp[:, 1, :],
                    kT[64:128, dt, kt * 128:(kt + 1) * 128],
                    qT[64:128, dt, q0:q0 + 512],
                    start=True, stop=True)
                if kt % 2 == 0:
                    e_pair = Ep.tile([128, 2, 2, 512], FP8, tag=f"E{qh}",
                                     name="e_pair")
                if (kt, qh) in D_SET:
                    nc.vector.tensor_scalar(
                        e_pair[:, kt % 2, :, :].bitcast(INT8), scp,
                        SC_L, SC_B, op0=MUL, op1=ADD)
                else:
                    nc.scalar.activation(
                        e_pair[:, kt % 2, :, :], scp, AF.Exp, scale=0.125)
                if kt % 2 == 1 or kt == kc - 1:
                    av_pair(cps, qh, kt, hA, hB, e_pair)
                if 0 < kt < kc - 1:
                    dummy_mm(cps[0], dummy_rhs)
                    dummy_mm(cps[1], dummy_rhs)
            drain(cps[0], 0, dt, q0)
            drain(cps[1], 1, dt, q0)

        for st in range(kc):
            vproj(st)
        project(wq_sb, qT, xT_sb, 0, q_tiles)
        project(wk_sb, kT, xcT_sb, 0, k_tiles)
        for dt in range(DT):
            chain(dt, 0)
            if dt + 1 < DT:
                project(wq_sb, qT, xT_sb, dt + 1, q_tiles)
            chain(dt, 1)
            if dt + 1 < DT:
                project(wk_sb, kT, xcT_sb, dt + 1, k_tiles)

    # ---------------- phase 3: out proj + LayerNorm ----------------
    with (
        tc.tile_pool(name="epi", bufs=3) as epi,
        tc.tile_pool(name="tke", bufs=1) as tke,
        tc.tile_pool(name="pjps", bufs=2, space="PSUM") as pjps,
    ):
        t_tiles = []
        mvall = tke.tile([128, ST, 2], F32, tag="mva", name="mva")
        for qt in range(ST):
            x_t = x_sb[:, qt, :]
            t = tke.tile([128, D], F32, tag=f"tq{qt}", name="t")
            t_tiles.append(t)
            for et in range(2):
                pps = pjps.tile([128, 512], F32, tag=f"pj{et}", name="pps")
                for dp in range(0, DT, 2):
                    nc.tensor.matmul(
                        pps,
                        ctxT[:, dp:dp + 2, qt * 128:(qt + 1) * 128],
                        wo_sb[:, dp:dp + 2, et * 512:(et + 1) * 512],
                        start=(dp == 0), stop=(dp == DT - 2),
                        perf_mode=DR)
                nc.vector.scalar_tensor_tensor(
                    out=t[:, et * 512:(et + 1) * 512],
                    in0=pps, scalar=IWS,
                    in1=x_t[:, et * 512:(et + 1) * 512],
                    op0=MUL, op1=ADD)
            stats = epi.tile([128, 2, nc.vector.BN_STATS_DIM], F32,
                             tag="stats", name="stats")
            tg = t.rearrange("p (g d) -> p g d", g=2)
            for g in range(2):
                nc.vector.bn_stats(stats[:, g, :], tg[:, g, :])
            nc.vector.bn_aggr(mvall[:, qt, :], stats)

        # batched Newton rsqrt on DVE: rstd = 1/sqrt(var + eps)
        vv = tke.tile([128, ST], F32, tag="nwv", name="nwv")
        nc.vector.tensor_scalar(vv, mvall[:, :, 1], EPS, None, op0=ADD)
        y = tke.tile([128, ST], F32, tag="nwy", name="nwy")
        nc.vector.tensor_scalar(y.bitcast(INT32), vv.bitcast(INT32),
                                1, None, op0=RSH)
        nc.vector.tensor_scalar(y.bitcast(INT32), y.bitcast(INT32),
                                -1, RSQRT_C, op0=MUL, op1=ADD)
        t1 = tke.tile([128, ST], F32, tag="nwt", name="nwt")
        for _ in range(2):  # two Newton iterations
            nc.vector.tensor_mul(t1, vv, y)
            nc.vector.tensor_mul(t1, t1, y)
            nc.vector.tensor_scalar(t1, t1, -0.5, 1.5, op0=MUL, op1=ADD)
            nc.vector.tensor_mul(y, y, t1)
        nmr = tke.tile([128, ST], F32, tag="nwm", name="nwm")
        nc.vector.scalar_tensor_tensor(
            out=nmr, in0=mvall[:, :, 0], scalar=-1.0, in1=y,
            op0=MUL, op1=MUL)

        for qt in range(ST):
            o_t = epi.tile([128, D], F32, tag="ot", name="o_t")
            nc.gpsimd.tensor_scalar(
                o_t, t_tiles[qt], y[:, qt:qt + 1], nmr[:, qt:qt + 1],
                op0=MUL, op1=ADD)
            nc.sync.dma_start(
                out=out[qt * 128:(qt + 1) * 128, :], in_=o_t)


def build_bass(reps=1, kc=8):
    nc = bacc.Bacc("TRN2", target_bir_lowering=False, debug=False)

    SC = kc * 128
    x32 = nc.dram_tensor("x32", [S, D], F32, kind="ExternalInput").ap()
    xT8 = nc.dram_tensor("xT8", [128, DT, S], FP8, kind="ExternalInput").ap()
    xcT8 = nc.dram_tensor("xcT8", [128, DT, SC], FP8,
                          kind="ExternalInput").ap()
    wq8 = nc.dram_tensor("wq8", [128, DT, D], FP8, kind="ExternalInput").ap()
    wk8 = nc.dram_tensor("wk8", [128, DT, D], FP8, kind="ExternalInput").ap()
    wv8 = nc.dram_tensor("wv8", [128, DT, D], FP8, kind="ExternalInput").ap()
    wo8 = nc.dram_tensor("wo8", [128, DT, D], FP8, kind="ExternalInput").ap()
    vmask = nc.dram_tensor("vmask", [128, kc], F32, kind="ExternalInput").ap()
    vones = nc.dram_tensor("vones", [128, kc, H], FP8,
                           kind="ExternalInput").ap()
    out = nc.dram_tensor("out", [S, D], F32, kind="ExternalOutput").ap()
    io = (x32, out)

    with tile.TileContext(nc) as tc:
        with tc.tile_pool(name="const", bufs=1) as const:
            vmask_sb = const.tile([128, kc], F32, name="vmask_sb")
            nc.sync.dma_start(out=vmask_sb, in_=vmask)
            # invocation-constant inputs: loaded once, resident in SBUF
            xT_sb = const.tile([128, DT, S], FP8, name="xT_sb")
            nc.sync.dma_start(out=xT_sb, in_=xT8)
            xcT_sb = const.tile([128, DT, SC], FP8, name="xcT_sb")
            nc.sync.dma_start(out=xcT_sb, in_=xcT8)
            wq_sb = const.tile([128, DT, D], FP8, name="wq_sb")
            nc.sync.dma_start(out=wq_sb, in_=wq8)
            wk_sb = const.tile([128, DT, D], FP8, name="wk_sb")
            nc.sync.dma_start(out=wk_sb, in_=wk8)
            wv_sb = const.tile([128, DT, D], FP8, name="wv_sb")
            nc.sync.dma_start(out=wv_sb, in_=wv8)
            wo_sb = const.tile([128, DT, D], FP8, name="wo_sb")
            nc.sync.dma_start(out=wo_sb, in_=wo8)
            x_sb = const.tile([128, ST, D], F32, name="x_sb")
            nc.sync.dma_start(out=x_sb,
                              in_=x32.rearrange("(t p) d -> p t d", p=128))
            qT = const.tile([128, DT, S], FP8, name="qT")
            kT = const.tile([128, DT, SC], FP8, name="kT")
            vp = const.tile([128, kc + 1, H, VST], FP8, name="vp")
            ctxT = const.tile([128, DT, S], FP8, name="ctxT")
            # ones column of v' (col 64 of each head slot): 1.0 for live
            # keys, 0.0 for masked tail keys (this applies the mask)
            nc.sync.dma_start(out=vp[:, 0:kc, :, 64], in_=vones)
            # zeroed pad key-plane: lets the odd-kc AV tail run DoubleRow
            # with a garbage E plane
            nc.vector.memset(vp[:, kc, :, :], 0.0)
            cst = (vmask_sb, qT, kT, vp, ctxT, xT_sb, xcT_sb,
                   wq_sb, wk_sb, wv_sb, wo_sb, x_sb)
            for _ in range(reps):
                _emit_body(nc, tc, io, cst, kc)

    nc.compile()
    return nc


_NC_CACHE = {}


def _get_nc(reps=1, kc=8):
    if (reps, kc) not in _NC_CACHE:
        _NC_CACHE[(reps, kc)] = build_bass(reps, kc)
    return _NC_CACHE[(reps, kc)]


def _pack_w(w):
    # [D, D] -> [128, DT, D] fp8 with w8[p, t, n] = w[t*128+p, n] * WS
    return np.ascontiguousarray(
        (np.asarray(w, np.float32) * WS).reshape(DT, 128, D)
        .transpose(1, 0, 2)).astype(ml_dtypes.float8_e4m3)


def _pack_xT(x):
    # [S', D] -> [128, DT, S'] fp8 with xT8[p, t, s] = x[s, t*128+p]
    return np.ascontiguousarray(
        np.asarray(x, np.float32).T.reshape(DT, 128, -1)
        .transpose(1, 0, 2)).astype(ml_dtypes.float8_e4m3)


def make_in_maps(x, mask, wq, bq, wk, bk, wv, bv, wo, bo, gamma, beta):
    for b in (bq, bk, bv, bo):
        assert not np.any(np.asarray(b)), "nonzero bias unsupported"
    x = np.asarray(x, dtype=np.float32)
    mask = np.asarray(mask)
    n_un_all = (mask == 0).sum(axis=1)
    kc = min(max((int(n_un_all.max()) + 127) // 128, 2), ST)
    SC = kc * 128
    idxs = [np.argsort(mask[c], kind="stable")[:SC] for c in range(B)]
    key_idx = np.arange(SC).reshape(kc, 128).T  # [128, kc] global key index
    common = {
        "wq8": _pack_w(wq), "wk8": _pack_w(wk),
        "wv8": _pack_w(wv), "wo8": _pack_w(wo),
    }
    maps = []
    for c in range(B):
        xc = x[c][idxs[c]]
        live = (key_idx < int(n_un_all[c]))  # [128, kc]
        maps.append(dict(
            common,
            x32=np.ascontiguousarray(x[c]),
            xT8=_pack_xT(x[c]),
            xcT8=_pack_xT(xc),
            vmask=np.ascontiguousarray(live.astype(np.float32) * IWS),
            vones=np.ascontiguousarray(
                np.broadcast_to(live[:, :, None], (128, kc, H))
                .astype(ml_dtypes.float8_e4m3))))
    return maps, kc


def kernel(x, mask, wq, bq, wk, bk, wv, bv, wo, bo, gamma, beta):
    in_maps, kc = make_in_maps(x, mask, wq, bq, wk, bk, wv, bv, wo, bo,
                               gamma, beta)
    nc = _get_nc(1, kc)
    last_err = None
    for _ in range(3):
        try:
            res = run_bass_kernel_spmd(nc, in_maps, core_ids=list(range(B)))
            outv = np.stack([res.results[c]["out"] for c in range(B)], axis=0)
            gamma = np.asarray(gamma, np.float32)
            beta = np.asarray(beta, np.float32)
            if not (np.all(gamma == 1.0) and np.all(beta == 0.0)):
                outv = outv * gamma + beta
            return outv
        except Exception as e:  # transient NRT device errors: retry
            last_err = e
            time.sleep(5)
    raise last_err


# revision 14
# speedup vs baseline: 1.2260x; 1.1522x over previous
"""Multi-head attention block on 8 Trainium2 NeuronCores, data-parallel over
batch, fp8 (e4m3) matmul datapath with DoubleRow perf mode.

Shapes (hardcoded): B=8, S=1024, H=16, HD=64, D=1024. One batch element per
core. Host pre-transposes/casts x and the weights to fp8 (weights scaled by
64 for fp8 range; the 1/64 is folded into the movers), and compacts keys
(unmasked first). Masking is applied by ZEROING the v'/ones rows of masked
(tail) keys, so the exp needs no per-key bias and can run on either engine.

Per-core dataflow v2:
  phase 1: qT/kT (fp8) and v' ([128keys, kc, H, 72] + ones col at 64, tail
           rows zeroed) via DoubleRow fp8 matmuls; movers on ScalarE
           (Identity activation, scale=1/64 or the per-key v-mask).
  phase 2 per dt (2 heads), qh outer: per kt ONE row-tiled matmul PAIR
           computes both heads' scoresT[k,q] concurrently (head A in PE rows
           0-63, head B in rows 64-127, separate PSUM banks); E = exp(s/8)
           computed per (kt,qh) over [128, 2head*512] either on ScalarE
           (Exp) or on VectorE via a Schraudolph int8 bit-trick that emits
           fp8 bits directly; ctx' psum [65, 512] per head accumulates
           v'^T @ E DoubleRow over kt pairs (row 64 = denominator).
           Normalize: DVE reciprocal of the den row, GpSimd
           partition_broadcast, DVE mult+fp8-cast into ctxT.
  phase 3: out proj fp8 DoubleRow; t = x + proj/64 (DVE STT); LayerNorm
           stats via bn_stats/bn_aggr, rstd via batched DVE Newton rsqrt
           (no ScalarE table switch!); final apply on GpSimd.
"""
import sys
import time

sys.path.insert(0, "/opt/trn_rl_repo")

import numpy as np
import ml_dtypes

import concourse.bass as bass
import concourse.bacc as bacc
import concourse.tile as tile
from concourse import mybir
from concourse.bass_utils import run_bass_kernel_spmd

F32 = mybir.dt.float32
FP8 = mybir.dt.float8e4
INT32 = mybir.dt.int32
INT8 = mybir.dt.int8
AF = mybir.ActivationFunctionType
DR = mybir.MatmulPerfMode.DoubleRow
MUL = mybir.AluOpType.mult
ADD = mybir.AluOpType.add
RSH = mybir.AluOpType.logical_shift_right

B, S, H, HD = 8, 1024, 16, 64
D = H * HD
EPS = 1e-6
ST = S // 128    # 8 s-tiles
DT = D // 128    # 8 d-tiles
NH = S // 512    # 2 query halves
WS = 64.0        # weight scale for fp8
IWS = 1.0 / WS
VST = 72         # per-head stride in v' (64 v dims + ones col + pad)
SC_L = 11.5416 * 0.125   # Schraudolph exp: 8*log2(e) * score scale
SC_B = 55.654            # bias: 7*8 - 8*log2(1.0308) (round-to-nearest)
RSQRT_C = 0x5F3759DF     # Schraudolph rsqrt seed constant
RECIP_C = 0x7EF311C3     # Schraudolph reciprocal constant

# exp-engine split: (kt, qh) pairs routed to the DVE Schraudolph path;
# everything else uses ScalarE Exp. Tuned for ACT/DVE load balance.
D_SET = {(2, 0), (2, 1), (4, 0)}


def _emit_body(nc, tc, io, cst, kc, pools):
    (x32, out) = io
    (vmask_sb, qT, kT, vp, ctxT, xT_sb, xcT_sb,
     wq_sb, wk_sb, wv_sb, wo_sb, x_sb) = cst
    SC = kc * 128
    kpairs = kc // 2          # full DoubleRow key-tile pairs for AV
    ktail = kc - 2 * kpairs   # 0 or 1 leftover key tile

    # ---------------- phases 1+2 merged: projections feed heads ------
    # PSUM tags: "scp" [128,2,512]x3 = 6 banks (scores ring, also carries
    # projection chunks); "ctx" [65,512]x2 = 2 banks (ctx accumulators,
    # also phase-3 out-proj chunks). Pools live across reps so phase-3
    # tails overlap the next rep.
    if True:
        (Ep, nrm, ps, scps, epi, tke) = pools

        def vproj(st):
            # v' natural [keys, dcols], scattered to head slots; the
            # per-key scale (1/64, or 0 for masked tail keys) applies
            # the mask. Both 512-col halves go into one scp-ring tile,
            # drained by a single merged mover.
            vps = scps.tile([128, 2, 512], F32, tag="scp", name="vps")
            for et in range(2):
                for kp in range(0, DT, 2):
                    nc.tensor.matmul(
                        vps[:, et, :],
                        xcT_sb[:, kp:kp + 2, st * 128:(st + 1) * 128],
                        wv_sb[:, kp:kp + 2, et * 512:(et + 1) * 512],
                        start=(kp == 0), stop=(kp == DT - 2),
                        perf_mode=DR)
            nc.scalar.activation(
                vp[:, st, :, 0:64],
                vps.rearrange("p e (h j) -> p (e h) j", j=64),
                AF.Identity, scale=vmask_sb[:, st:st + 1])

        k_tiles = [(i * 512, min(512, SC - i * 512))
                   for i in range((SC + 511) // 512)]
        q_tiles = [(i * 512, 512) for i in range(NH)]

        def project(wsb_t, dstT, rhsT, mg, ntiles):
            qps = scps.tile([128, 2, 512], F32, tag="scp", name="qps")
            tot = 0
            for ci, (n0, nsz) in enumerate(ntiles):
                for kp in range(0, DT, 2):
                    nc.tensor.matmul(
                        qps[:, ci, 0:nsz],
                        wsb_t[:, kp:kp + 2, mg * 128:(mg + 1) * 128],
                        rhsT[:, kp:kp + 2, n0:n0 + nsz],
                        start=(kp == 0), stop=(kp == DT - 2),
                        perf_mode=DR)
                tot += nsz
            nc.scalar.activation(
                dstT[:, mg, 0:tot],
                qps.rearrange("p a b -> p (a b)")[:, 0:tot],
                AF.Identity, scale=IWS)

        def drain(c, hi, dt, q0):
            # normalize: Schraudolph recip of den row (DVE), partition
            # broadcast (GpSimd), multiply + fp8 cast into ctxT (DVE)
            base = hi * 64
            r0 = nrm.tile([1, 512], F32, tag=f"r0{hi}", name="r0")
            nc.vector.tensor_scalar(
                r0.bitcast(INT32), c[64:65, :].bitcast(INT32),
                -1, RECIP_C, op0=MUL, op1=ADD)
            rbc = nrm.tile([64, 512], F32, tag=f"rbc{hi}", name="rbc")
            nc.gpsimd.partition_broadcast(rbc, r0)
            nc.vector.tensor_mul(
                ctxT[base:base + 64, dt, q0:q0 + 512],
                c[0:64, :], rbc)

        def av_pair(cps2, qh, kt, hA, hB, e_pair):
            # DoubleRow AV over the key-tile pair ending at odd kt, or
            # the stride-0 pad trick for an odd-kc tail at even kt.
            last_pair = (kt % 2 == 1 and ktail == 0 and kt == kc - 1)
            if kt % 2 == 1:
                for hi, h in ((0, hA), (1, hB)):
                    nc.tensor.matmul(
                        cps2[hi],
                        vp[:, kt - 1:kt + 1, h, 0:65],
                        e_pair[:, 0:2, hi, :],
                        start=(kt == 1), stop=last_pair,
                        perf_mode=DR)
            else:
                for hi, h in ((0, hA), (1, hB)):
                    e0 = e_pair[:, 0, hi, :]
                    e00 = bass.AP(
                        tensor=e0.tensor, offset=e0.offset,
                        ap=[list(e0.ap[0]), [0, 2]]
                        + [list(a) for a in e0.ap[1:]])
                    nc.tensor.matmul(
                        cps2[hi],
                        vp[:, kt:kt + 2, h, 0:65],
                        e00,
                        start=(kc == 1), stop=True,
                        perf_mode=DR)

        def dummy_mm(c, rhs_view):
            # HAM keepalive: accumulate exact zeros (vp pad plane,
            # stride-0 pair) into a live ctx psum so the PE array never
            # idles long enough to re-throttle to K=4/8.
            z0 = vp[:, kc, 0, 0:65]
            z00 = bass.AP(
                tensor=z0.tensor, offset=z0.offset,
                ap=[list(z0.ap[0]), [0, 2]] + [list(a) for a in z0.ap[1:]])
            nc.tensor.matmul(c, z00, rhs_view, start=False, stop=False,
                             perf_mode=DR, skip_group_check=True)

        def chain(dt, qh):
            """Attention for heads (2dt, 2dt+1) over query half qh."""
            hA, hB = 2 * dt, 2 * dt + 1
            q0 = qh * 512
            cps = [ps.tile([65, 512], F32, tag="ctx", name="cps")
                   for _ in range(2)]
            dummy_rhs = qT[:, dt, 0:1024].rearrange("p (a b) -> p a b", a=2)
            e_pair = None
            for kt in range(kc):
                scp = scps.tile([128, 2, 512], F32, tag="scp", name="scp")
                # row-tiled concurrent pair: head A rows 0-63, head B
                # rows 64-127 of the PE array
                nc.tensor.matmul(
                    scp[:, 0, :],
                    kT[0:64, dt, kt * 128:(kt + 1) * 128],
                    qT[0:64, dt, q0:q0 + 512],
                    start=True, stop=True)
                nc.tensor.matmul(
                    scp[:, 1, :],
                    kT[64:128, dt, kt * 128:(kt + 1) * 128],
                    qT[64:128, dt, q0:q0 + 512],
                    start=True, stop=True)
                if kt % 2 == 0:
                    e_pair = Ep.tile([128, 2, 2, 512], FP8, tag=f"E{qh}",
                                     name="e_pair")
                if (kt, qh) in D_SET:
                    nc.vector.tensor_scalar(
                        e_pair[:, kt % 2, :, :].bitcast(INT8), scp,
                        SC_L, SC_B, op0=MUL, op1=ADD)
                else:
                    nc.scalar.activation(
                        e_pair[:, kt % 2, :, :], scp, AF.Exp, scale=0.125)
                if kt % 2 == 1 or kt == kc - 1:
                    av_pair(cps, qh, kt, hA, hB, e_pair)
                if 0 < kt < kc - 1:
                    dummy_mm(cps[0], dummy_rhs)
                    dummy_mm(cps[1], dummy_rhs)
            drain(cps[0], 0, dt, q0)
            drain(cps[1], 1, dt, q0)

        for st in range(kc):
            vproj(st)
        project(wq_sb, qT, xT_sb, 0, q_tiles)
        project(wk_sb, kT, xcT_sb, 0, k_tiles)
        for dt in range(DT):
            chain(dt, 0)
            if dt + 1 < DT:
                project(wq_sb, qT, xT_sb, dt + 1, q_tiles)
            chain(dt, 1)
            if dt + 1 < DT:
                project(wk_sb, kT, xcT_sb, dt + 1, k_tiles)

    # ---------------- phase 3: out proj + LayerNorm ----------------
    if True:
        t_tiles = []
        mvall = tke.tile([128, ST, 2], F32, tag="mva", name="mva")
        for qt in range(ST):
            x_t = x_sb[:, qt, :]
            t = tke.tile([128, D], F32, tag=f"tq{qt}", name="t")
            t_tiles.append(t)
            for et in range(2):
                pps = ps.tile([128, 512], F32, tag="ctx", name="pps")
                for dp in range(0, DT, 2):
                    nc.tensor.matmul(
                        pps,
                        ctxT[:, dp:dp + 2, qt * 128:(qt + 1) * 128],
                        wo_sb[:, dp:dp + 2, et * 512:(et + 1) * 512],
                        start=(dp == 0), stop=(dp == DT - 2),
                        perf_mode=DR)
                nc.vector.scalar_tensor_tensor(
                    out=t[:, et * 512:(et + 1) * 512],
                    in0=pps, scalar=IWS,
                    in1=x_t[:, et * 512:(et + 1) * 512],
                    op0=MUL, op1=ADD)
            stats = epi.tile([128, 2, nc.vector.BN_STATS_DIM], F32,
                             tag="stats", name="stats")
            tg = t.rearrange("p (g d) -> p g d", g=2)
            for g in range(2):
                nc.vector.bn_stats(stats[:, g, :], tg[:, g, :])
            nc.vector.bn_aggr(mvall[:, qt, :], stats)

        # batched Newton rsqrt on DVE: rstd = 1/sqrt(var + eps)
        vv = tke.tile([128, ST], F32, tag="nwv", name="nwv")
        nc.vector.tensor_scalar(vv, mvall[:, :, 1], EPS, None, op0=ADD)
        y = tke.tile([128, ST], F32, tag="nwy", name="nwy")
        nc.vector.tensor_scalar(y.bitcast(INT32), vv.bitcast(INT32),
                                1, None, op0=RSH)
        nc.vector.tensor_scalar(y.bitcast(INT32), y.bitcast(INT32),
                                -1, RSQRT_C, op0=MUL, op1=ADD)
        t1 = tke.tile([128, ST], F32, tag="nwt", name="nwt")
        for _ in range(2):  # two Newton iterations
            nc.vector.tensor_mul(t1, vv, y)
            nc.vector.tensor_mul(t1, t1, y)
            nc.vector.tensor_scalar(t1, t1, -0.5, 1.5, op0=MUL, op1=ADD)
            nc.vector.tensor_mul(y, y, t1)
        nmr = tke.tile([128, ST], F32, tag="nwm", name="nwm")
        nc.vector.scalar_tensor_tensor(
            out=nmr, in0=mvall[:, :, 0], scalar=-1.0, in1=y,
            op0=MUL, op1=MUL)

        for qt in range(ST):
            o_t = epi.tile([128, D], F32, tag="ot", name="o_t")
            nc.gpsimd.tensor_scalar(
                o_t, t_tiles[qt], y[:, qt:qt + 1], nmr[:, qt:qt + 1],
                op0=MUL, op1=ADD)
            nc.sync.dma_start(
                out=out[qt * 128:(qt + 1) * 128, :], in_=o_t)


def build_bass(reps=1, kc=8):
    nc = bacc.Bacc("TRN2", target_bir_lowering=False, debug=False)

    SC = kc * 128
    x32 = nc.dram_tensor("x32", [S, D], F32, kind="ExternalInput").ap()
    xT8 = nc.dram_tensor("xT8", [128, DT, S], FP8, kind="ExternalInput").ap()
    xcT8 = nc.dram_tensor("xcT8", [128, DT, SC], FP8,
                          kind="ExternalInput").ap()
    wq8 = nc.dram_tensor("wq8", [128, DT, D], FP8, kind="ExternalInput").ap()
    wk8 = nc.dram_tensor("wk8", [128, DT, D], FP8, kind="ExternalInput").ap()
    wv8 = nc.dram_tensor("wv8", [128, DT, D], FP8, kind="ExternalInput").ap()
    wo8 = nc.dram_tensor("wo8", [128, DT, D], FP8, kind="ExternalInput").ap()
    vmask = nc.dram_tensor("vmask", [128, kc], F32, kind="ExternalInput").ap()
    vones = nc.dram_tensor("vones", [128, kc, H], FP8,
                           kind="ExternalInput").ap()
    out = nc.dram_tensor("out", [S, D], F32, kind="ExternalOutput").ap()
    io = (x32, out)

    with tile.TileContext(nc) as tc:
        with tc.tile_pool(name="const", bufs=1) as const:
            vmask_sb = const.tile([128, kc], F32, name="vmask_sb")
            nc.sync.dma_start(out=vmask_sb, in_=vmask)
            # invocation-constant inputs: loaded once, resident in SBUF
            xT_sb = const.tile([128, DT, S], FP8, name="xT_sb")
            nc.sync.dma_start(out=xT_sb, in_=xT8)
            xcT_sb = const.tile([128, DT, SC], FP8, name="xcT_sb")
            nc.sync.dma_start(out=xcT_sb, in_=xcT8)
            wq_sb = const.tile([128, DT, D], FP8, name="wq_sb")
            nc.sync.dma_start(out=wq_sb, in_=wq8)
            wk_sb = const.tile([128, DT, D], FP8, name="wk_sb")
            nc.sync.dma_start(out=wk_sb, in_=wk8)
            wv_sb = const.tile([128, DT, D], FP8, name="wv_sb")
            nc.sync.dma_start(out=wv_sb, in_=wv8)
            wo_sb = const.tile([128, DT, D], FP8, name="wo_sb")
            nc.sync.dma_start(out=wo_sb, in_=wo8)
            x_sb = const.tile([128, ST, D], F32, name="x_sb")
            nc.sync.dma_start(out=x_sb,
                              in_=x32.rearrange("(t p) d -> p t d", p=128))
            qT = const.tile([128, DT, S], FP8, name="qT")
            kT = const.tile([128, DT, SC], FP8, name="kT")
            vp = const.tile([128, kc + 1, H, VST], FP8, name="vp")
            ctxT = const.tile([128, DT, S], FP8, name="ctxT")
            # ones column of v' (col 64 of each head slot): 1.0 for live
            # keys, 0.0 for masked tail keys (this applies the mask)
            nc.sync.dma_start(out=vp[:, 0:kc, :, 64], in_=vones)
            # zeroed pad key-plane: lets the odd-kc AV tail run DoubleRow
            # with a garbage E plane
            nc.vector.memset(vp[:, kc, :, :], 0.0)
            cst = (vmask_sb, qT, kT, vp, ctxT, xT_sb, xcT_sb,
                   wq_sb, wk_sb, wv_sb, wo_sb, x_sb)
            with (
                tc.tile_pool(name="Ep", bufs=4) as Ep,
                tc.tile_pool(name="nrm", bufs=2) as nrm,
                tc.tile_pool(name="ps", bufs=2, space="PSUM") as ps,
                tc.tile_pool(name="scp", bufs=3, space="PSUM") as scps,
                tc.tile_pool(name="epi", bufs=3) as epi,
                tc.tile_pool(name="tke", bufs=1) as tke,
            ):
                pools = (Ep, nrm, ps, scps, epi, tke)
                for _ in range(reps):
                    _emit_body(nc, tc, io, cst, kc, pools)

    nc.compile()
    return nc


_NC_CACHE = {}


def _get_nc(reps=1, kc=8):
    if (reps, kc) not in _NC_CACHE:
        _NC_CACHE[(reps, kc)] = build_bass(reps, kc)
    return _NC_CACHE[(reps, kc)]


def _pack_w(w):
    # [D, D] -> [128, DT, D] fp8 with w8[p, t, n] = w[t*128+p, n] * WS
    return np.ascontiguousarray(
        (np.asarray(w, np.float32) * WS).reshape(DT, 128, D)
        .transpose(1, 0, 2)).astype(ml_dtypes.float8_e4m3)


def _pack_xT(x):
    # [S', D] -> [128, DT, S'] fp8 with xT8[p, t, s] = x[s, t*128+p]
    return np.ascontiguousarray(
        np.asarray(x, np.float32).T.reshape(DT, 128, -1)
        .transpose(1, 0, 2)).astype(ml_dtypes.float8_e4m3)


def make_in_maps(x, mask, wq, bq, wk, bk, wv, bv, wo, bo, gamma, beta):
    for b in (bq, bk, bv, bo):
        assert not np.any(np.asarray(b)), "nonzero bias unsupported"
    x = np.asarray(x, dtype=np.float32)
    mask = np.asarray(mask)
    n_un_all = (mask == 0).sum(axis=1)
    kc = min(max((int(n_un_all.max()) + 127) // 128, 2), ST)
    SC = kc * 128
    idxs = [np.argsort(mask[c], kind="stable")[:SC] for c in range(B)]
    key_idx = np.arange(SC).reshape(kc, 128).T  # [128, kc] global key index
    common = {
        "wq8": _pack_w(wq), "wk8": _pack_w(wk),
        "wv8": _pack_w(wv), "wo8": _pack_w(wo),
    }
    maps = []
    for c in range(B):
        xc = x[c][idxs[c]]
        live = (key_idx < int(n_un_all[c]))  # [128, kc]
        maps.append(dict(
            common,
            x32=np.ascontiguousarray(x[c]),
            xT8=_pack_xT(x[c]),
            xcT8=_pack_xT(xc),
            vmask=np.ascontiguousarray(live.astype(np.float32) * IWS),
            vones=np.ascontiguousarray(
                np.broadcast_to(live[:, :, None], (128, kc, H))
                .astype(ml_dtypes.float8_e4m3))))
    return maps, kc


def kernel(x, mask, wq, bq, wk, bk, wv, bv, wo, bo, gamma, beta):
    in_maps, kc = make_in_maps(x, mask, wq, bq, wk, bk, wv, bv, wo, bo,
                               gamma, beta)
    nc = _get_nc(1, kc)
    last_err = None
    for _ in range(3):
        try:
            res = run_bass_kernel_spmd(nc, in_maps, core_ids=list(range(B)))
            outv = np.stack([res.results[c]["out"] for c in range(B)], axis=0)
            gamma = np.asarray(gamma, np.float32)
            beta = np.asarray(beta, np.float32)
            if not (np.all(gamma == 1.0) and np.all(beta == 0.0)):
                outv = outv * gamma + beta
            return outv
        except Exception as e:  # transient NRT device errors: retry
            last_err = e
            time.sleep(5)
    raise last_err


# revision 15
# speedup vs baseline: 1.2819x; 1.0455x over previous
"""Multi-head attention block on 8 Trainium2 NeuronCores, data-parallel over
batch, fp8 (e4m3) matmul datapath with DoubleRow perf mode.

Shapes (hardcoded): B=8, S=1024, H=16, HD=64, D=1024. One batch element per
core. Host pre-transposes/casts x and the weights to fp8 (weights scaled by
64 for fp8 range; the 1/64 is folded into the movers), and compacts keys
(unmasked first). Masking is applied by ZEROING the v'/ones rows of masked
(tail) keys, so the exp needs no per-key bias and can run on either engine.

Per-core dataflow v2:
  phase 1: qT/kT (fp8) and v' ([128keys, kc, H, 72] + ones col at 64, tail
           rows zeroed) via DoubleRow fp8 matmuls; movers on ScalarE
           (Identity activation, scale=1/64 or the per-key v-mask).
  phase 2 per dt (2 heads), qh outer: per kt ONE row-tiled matmul PAIR
           computes both heads' scoresT[k,q] concurrently (head A in PE rows
           0-63, head B in rows 64-127, separate PSUM banks); E = exp(s/8)
           computed per (kt,qh) over [128, 2head*512] either on ScalarE
           (Exp) or on VectorE via a Schraudolph int8 bit-trick that emits
           fp8 bits directly; ctx' psum [65, 512] per head accumulates
           v'^T @ E DoubleRow over kt pairs (row 64 = denominator).
           Normalize: DVE reciprocal of the den row, GpSimd
           partition_broadcast, DVE mult+fp8-cast into ctxT.
  phase 3: out proj fp8 DoubleRow; t = x + proj/64 (DVE STT); LayerNorm
           stats via bn_stats/bn_aggr, rstd via batched DVE Newton rsqrt
           (no ScalarE table switch!); final apply on GpSimd.
"""
import sys
import time

sys.path.insert(0, "/opt/trn_rl_repo")

import numpy as np
import ml_dtypes

import concourse.bass as bass
import concourse.bacc as bacc
import concourse.tile as tile
from concourse import mybir
from concourse.bass_utils import run_bass_kernel_spmd

F32 = mybir.dt.float32
FP8 = mybir.dt.float8e4
INT32 = mybir.dt.int32
INT8 = mybir.dt.int8
AF = mybir.ActivationFunctionType
DR = mybir.MatmulPerfMode.DoubleRow
MUL = mybir.AluOpType.mult
ADD = mybir.AluOpType.add
RSH = mybir.AluOpType.logical_shift_right

B, S, H, HD = 8, 1024, 16, 64
D = H * HD
EPS = 1e-6
ST = S // 128    # 8 s-tiles
DT = D // 128    # 8 d-tiles
NH = S // 512    # 2 query halves
WS = 64.0        # weight scale for fp8
IWS = 1.0 / WS
VST = 72         # per-head stride in v' (64 v dims + ones col + pad)
SC_L = 11.5416 * 0.125   # Schraudolph exp: 8*log2(e) * score scale
SC_B = 55.654            # bias: 7*8 - 8*log2(1.0308) (round-to-nearest)
RSQRT_C = 0x5F3759DF     # Schraudolph rsqrt seed constant
RECIP_C = 0x7EF311C3     # Schraudolph reciprocal constant

# exp-engine split: (kt, qh) pairs routed to the DVE Schraudolph path;
# everything else uses ScalarE Exp. Tuned for ACT/DVE load balance.
D_SET = {(2, 0), (2, 1)}


def _emit_body(nc, tc, io, cst, kc, pools):
    (x32, out) = io
    (vmask_sb, qT, kT, vp, ctxT, xT_sb, xcT_sb,
     wq_sb, wk_sb, wv_sb, wo_sb, x_sb) = cst
    SC = kc * 128
    kpairs = kc // 2          # full DoubleRow key-tile pairs for AV
    ktail = kc - 2 * kpairs   # 0 or 1 leftover key tile

    # ---------------- phases 1+2 merged: projections feed heads ------
    # PSUM tags: "scp" [128,2,512]x3 = 6 banks (scores ring, also carries
    # projection chunks); "ctx" [65,512]x2 = 2 banks (ctx accumulators,
    # also phase-3 out-proj chunks). Pools live across reps so phase-3
    # tails overlap the next rep.
    if True:
        (Ep, nrm, ps, scps, epi, tke) = pools

        def vproj(st):
            # v' natural [keys, dcols], scattered to head slots; the
            # per-key scale (1/64, or 0 for masked tail keys) applies
            # the mask. Both 512-col halves go into one scp-ring tile,
            # drained by a single merged mover.
            vps = scps.tile([128, 2, 512], F32, tag="scp", name="vps")
            for et in range(2):
                for kp in range(0, DT, 2):
                    nc.tensor.matmul(
                        vps[:, et, :],
                        xcT_sb[:, kp:kp + 2, st * 128:(st + 1) * 128],
                        wv_sb[:, kp:kp + 2, et * 512:(et + 1) * 512],
                        start=(kp == 0), stop=(kp == DT - 2),
                        perf_mode=DR)
            nc.scalar.activation(
                vp[:, st, :, 0:64],
                vps.rearrange("p e (h j) -> p (e h) j", j=64),
                AF.Identity, scale=vmask_sb[:, st:st + 1])

        k_tiles = [(i * 512, min(512, SC - i * 512))
                   for i in range((SC + 511) // 512)]
        q_tiles = [(i * 512, 512) for i in range(NH)]

        def project(wsb_t, dstT, rhsT, mg, ntiles):
            qps = scps.tile([128, 2, 512], F32, tag="scp", name="qps")
            tot = 0
            for ci, (n0, nsz) in enumerate(ntiles):
                for kp in range(0, DT, 2):
                    nc.tensor.matmul(
                        qps[:, ci, 0:nsz],
                        wsb_t[:, kp:kp + 2, mg * 128:(mg + 1) * 128],
                        rhsT[:, kp:kp + 2, n0:n0 + nsz],
                        start=(kp == 0), stop=(kp == DT - 2),
                        perf_mode=DR)
                tot += nsz
            nc.scalar.activation(
                dstT[:, mg, 0:tot],
                qps.rearrange("p a b -> p (a b)")[:, 0:tot],
                AF.Identity, scale=IWS)

        def drain(c, hi, dt, q0):
            # normalize: Schraudolph recip of den row (DVE), partition
            # broadcast (GpSimd), multiply + fp8 cast into ctxT (DVE)
            base = hi * 64
            r0 = nrm.tile([1, 512], F32, tag=f"r0{hi}", name="r0")
            nc.vector.tensor_scalar(
                r0.bitcast(INT32), c[64:65, :].bitcast(INT32),
                -1, RECIP_C, op0=MUL, op1=ADD)
            rbc = nrm.tile([64, 512], F32, tag=f"rbc{hi}", name="rbc")
            nc.gpsimd.partition_broadcast(rbc, r0)
            nc.vector.tensor_mul(
                ctxT[base:base + 64, dt, q0:q0 + 512],
                c[0:64, :], rbc)

        def av_pair(cps2, qh, kt, hA, hB, e_pair):
            # DoubleRow AV over the key-tile pair ending at odd kt, or
            # the stride-0 pad trick for an odd-kc tail at even kt.
            last_pair = (kt % 2 == 1 and ktail == 0 and kt == kc - 1)
            if kt % 2 == 1:
                for hi, h in ((0, hA), (1, hB)):
                    nc.tensor.matmul(
                        cps2[hi],
                        vp[:, kt - 1:kt + 1, h, 0:65],
                        e_pair[:, 0:2, hi, :],
                        start=(kt == 1), stop=last_pair,
                        perf_mode=DR)
            else:
                for hi, h in ((0, hA), (1, hB)):
                    e0 = e_pair[:, 0, hi, :]
                    e00 = bass.AP(
                        tensor=e0.tensor, offset=e0.offset,
                        ap=[list(e0.ap[0]), [0, 2]]
                        + [list(a) for a in e0.ap[1:]])
                    nc.tensor.matmul(
                        cps2[hi],
                        vp[:, kt:kt + 2, h, 0:65],
                        e00,
                        start=(kc == 1), stop=True,
                        perf_mode=DR)

        def dummy_mm(c, rhs_view):
            # HAM keepalive: accumulate exact zeros (vp pad plane,
            # stride-0 pair) into a live ctx psum so the PE array never
            # idles long enough to re-throttle to K=4/8.
            z0 = vp[:, kc, 0, 0:65]
            z00 = bass.AP(
                tensor=z0.tensor, offset=z0.offset,
                ap=[list(z0.ap[0]), [0, 2]] + [list(a) for a in z0.ap[1:]])
            nc.tensor.matmul(c, z00, rhs_view, start=False, stop=False,
                             perf_mode=DR, skip_group_check=True)

        def chain(dt, qh):
            """Attention for heads (2dt, 2dt+1) over query half qh."""
            hA, hB = 2 * dt, 2 * dt + 1
            q0 = qh * 512
            cps = [ps.tile([65, 512], F32, tag="ctx", name="cps")
                   for _ in range(2)]
            dummy_rhs = qT[:, dt, 0:1024].rearrange("p (a b) -> p a b", a=2)
            e_pair = None
            for kt in range(kc):
                scp = scps.tile([128, 2, 512], F32, tag="scp", name="scp")
                # row-tiled concurrent pair: head A rows 0-63, head B
                # rows 64-127 of the PE array
                nc.tensor.matmul(
                    scp[:, 0, :],
                    kT[0:64, dt, kt * 128:(kt + 1) * 128],
                    qT[0:64, dt, q0:q0 + 512],
                    start=True, stop=True)
                nc.tensor.matmul(
                    scp[:, 1, :],
                    kT[64:128, dt, kt * 128:(kt + 1) * 128],
                    qT[64:128, dt, q0:q0 + 512],
                    start=True, stop=True)
                if kt % 2 == 0:
                    e_pair = Ep.tile([128, 2, 2, 512], FP8, tag=f"E{qh}",
                                     name="e_pair")
                if (kt, qh) in D_SET:
                    nc.vector.tensor_scalar(
                        e_pair[:, kt % 2, :, :].bitcast(INT8), scp,
                        SC_L, SC_B, op0=MUL, op1=ADD)
                else:
                    nc.scalar.activation(
                        e_pair[:, kt % 2, :, :], scp, AF.Exp, scale=0.125)
                if kt % 2 == 1 or kt == kc - 1:
                    av_pair(cps, qh, kt, hA, hB, e_pair)
                if 0 < kt < kc - 1:
                    dummy_mm(cps[kt % 2], dummy_rhs)
            drain(cps[0], 0, dt, q0)
            drain(cps[1], 1, dt, q0)

        for st in range(kc):
            vproj(st)
        project(wq_sb, qT, xT_sb, 0, q_tiles)
        project(wk_sb, kT, xcT_sb, 0, k_tiles)
        for dt in range(DT):
            chain(dt, 0)
            if dt + 1 < DT:
                project(wq_sb, qT, xT_sb, dt + 1, q_tiles)
            chain(dt, 1)
            if dt + 1 < DT:
                project(wk_sb, kT, xcT_sb, dt + 1, k_tiles)

    # ---------------- phase 3: out proj + LayerNorm ----------------
    if True:
        t_tiles = []
        mvall = tke.tile([128, ST, 2], F32, tag="mva", name="mva")
        for qt in range(ST):
            x_t = x_sb[:, qt, :]
            t = tke.tile([128, D], F32, tag=f"tq{qt}", name="t")
            t_tiles.append(t)
            for et in range(2):
                pps = ps.tile([128, 512], F32, tag="ctx", name="pps")
                for dp in range(0, DT, 2):
                    nc.tensor.matmul(
                        pps,
                        ctxT[:, dp:dp + 2, qt * 128:(qt + 1) * 128],
                        wo_sb[:, dp:dp + 2, et * 512:(et + 1) * 512],
                        start=(dp == 0), stop=(dp == DT - 2),
                        perf_mode=DR)
                nc.vector.scalar_tensor_tensor(
                    out=t[:, et * 512:(et + 1) * 512],
                    in0=pps, scalar=IWS,
                    in1=x_t[:, et * 512:(et + 1) * 512],
                    op0=MUL, op1=ADD)
            stats = epi.tile([128, 2, nc.vector.BN_STATS_DIM], F32,
                             tag="stats", name="stats")
            tg = t.rearrange("p (g d) -> p g d", g=2)
            for g in range(2):
                nc.vector.bn_stats(stats[:, g, :], tg[:, g, :])
            nc.vector.bn_aggr(mvall[:, qt, :], stats)

        # batched Newton rsqrt on DVE: rstd = 1/sqrt(var + eps)
        vv = tke.tile([128, ST], F32, tag="nwv", name="nwv")
        nc.vector.tensor_scalar(vv, mvall[:, :, 1], EPS, None, op0=ADD)
        y = tke.tile([128, ST], F32, tag="nwy", name="nwy")
        nc.vector.tensor_scalar(y.bitcast(INT32), vv.bitcast(INT32),
                                1, None, op0=RSH)
        nc.vector.tensor_scalar(y.bitcast(INT32), y.bitcast(INT32),
                                -1, RSQRT_C, op0=MUL, op1=ADD)
        t1 = tke.tile([128, ST], F32, tag="nwt", name="nwt")
        for _ in range(2):  # two Newton iterations
            nc.vector.tensor_mul(t1, vv, y)
            nc.vector.tensor_mul(t1, t1, y)
            nc.vector.tensor_scalar(t1, t1, -0.5, 1.5, op0=MUL, op1=ADD)
            nc.vector.tensor_mul(y, y, t1)
        nmr = tke.tile([128, ST], F32, tag="nwm", name="nwm")
        nc.vector.scalar_tensor_tensor(
            out=nmr, in0=mvall[:, :, 0], scalar=-1.0, in1=y,
            op0=MUL, op1=MUL)

        for qt in range(ST):
            o_t = epi.tile([128, D], F32, tag="ot", name="o_t")
            nc.gpsimd.tensor_scalar(
                o_t, t_tiles[qt], y[:, qt:qt + 1], nmr[:, qt:qt + 1],
                op0=MUL, op1=ADD)
            nc.sync.dma_start(
                out=out[qt * 128:(qt + 1) * 128, :], in_=o_t)


def build_bass(reps=1, kc=8):
    nc = bacc.Bacc("TRN2", target_bir_lowering=False, debug=False)

    SC = kc * 128
    x32 = nc.dram_tensor("x32", [S, D], F32, kind="ExternalInput").ap()
    xT8 = nc.dram_tensor("xT8", [128, DT, S], FP8, kind="ExternalInput").ap()
    xcT8 = nc.dram_tensor("xcT8", [128, DT, SC], FP8,
                          kind="ExternalInput").ap()
    wq8 = nc.dram_tensor("wq8", [128, DT, D], FP8, kind="ExternalInput").ap()
    wk8 = nc.dram_tensor("wk8", [128, DT, D], FP8, kind="ExternalInput").ap()
    wv8 = nc.dram_tensor("wv8", [128, DT, D], FP8, kind="ExternalInput").ap()
    wo8 = nc.dram_tensor("wo8", [128, DT, D], FP8, kind="ExternalInput").ap()
    vmask = nc.dram_tensor("vmask", [128, kc], F32, kind="ExternalInput").ap()
    vones = nc.dram_tensor("vones", [128, kc, H], FP8,
                           kind="ExternalInput").ap()
    out = nc.dram_tensor("out", [S, D], F32, kind="ExternalOutput").ap()
    io = (x32, out)

    with tile.TileContext(nc) as tc:
        with tc.tile_pool(name="const", bufs=1) as const:
            vmask_sb = const.tile([128, kc], F32, name="vmask_sb")
            nc.sync.dma_start(out=vmask_sb, in_=vmask)
            # invocation-constant inputs: loaded once, resident in SBUF
            xT_sb = const.tile([128, DT, S], FP8, name="xT_sb")
            nc.sync.dma_start(out=xT_sb, in_=xT8)
            xcT_sb = const.tile([128, DT, SC], FP8, name="xcT_sb")
            nc.sync.dma_start(out=xcT_sb, in_=xcT8)
            wq_sb = const.tile([128, DT, D], FP8, name="wq_sb")
            nc.sync.dma_start(out=wq_sb, in_=wq8)
            wk_sb = const.tile([128, DT, D], FP8, name="wk_sb")
            nc.sync.dma_start(out=wk_sb, in_=wk8)
            wv_sb = const.tile([128, DT, D], FP8, name="wv_sb")
            nc.sync.dma_start(out=wv_sb, in_=wv8)
            wo_sb = const.tile([128, DT, D], FP8, name="wo_sb")
            nc.sync.dma_start(out=wo_sb, in_=wo8)
            x_sb = const.tile([128, ST, D], F32, name="x_sb")
            nc.sync.dma_start(out=x_sb,
                              in_=x32.rearrange("(t p) d -> p t d", p=128))
            qT = const.tile([128, DT, S], FP8, name="qT")
            kT = const.tile([128, DT, SC], FP8, name="kT")
            vp = const.tile([128, kc + 1, H, VST], FP8, name="vp")
            ctxT = const.tile([128, DT, S], FP8, name="ctxT")
            # ones column of v' (col 64 of each head slot): 1.0 for live
            # keys, 0.0 for masked tail keys (this applies the mask)
            nc.sync.dma_start(out=vp[:, 0:kc, :, 64], in_=vones)
            # zeroed pad key-plane: lets the odd-kc AV tail run DoubleRow
            # with a garbage E plane
            nc.vector.memset(vp[:, kc, :, :], 0.0)
            cst = (vmask_sb, qT, kT, vp, ctxT, xT_sb, xcT_sb,
                   wq_sb, wk_sb, wv_sb, wo_sb, x_sb)
            with (
                tc.tile_pool(name="Ep", bufs=4) as Ep,
                tc.tile_pool(name="nrm", bufs=2) as nrm,
                tc.tile_pool(name="ps", bufs=2, space="PSUM") as ps,
                tc.tile_pool(name="scp", bufs=3, space="PSUM") as scps,
                tc.tile_pool(name="epi", bufs=3) as epi,
                tc.tile_pool(name="tke", bufs=1) as tke,
            ):
                pools = (Ep, nrm, ps, scps, epi, tke)
                for _ in range(reps):
                    _emit_body(nc, tc, io, cst, kc, pools)

    nc.compile()
    return nc


_NC_CACHE = {}


def _get_nc(reps=1, kc=8):
    if (reps, kc) not in _NC_CACHE:
        _NC_CACHE[(reps, kc)] = build_bass(reps, kc)
    return _NC_CACHE[(reps, kc)]


def _pack_w(w):
    # [D, D] -> [128, DT, D] fp8 with w8[p, t, n] = w[t*128+p, n] * WS
    return np.ascontiguousarray(
        (np.asarray(w, np.float32) * WS).reshape(DT, 128, D)
        .transpose(1, 0, 2)).astype(ml_dtypes.float8_e4m3)


def _pack_xT(x):
    # [S', D] -> [128, DT, S'] fp8 with xT8[p, t, s] = x[s, t*128+p]
    return np.ascontiguousarray(
        np.asarray(x, np.float32).T.reshape(DT, 128, -1)
        .transpose(1, 0, 2)).astype(ml_dtypes.float8_e4m3)


def make_in_maps(x, mask, wq, bq, wk, bk, wv, bv, wo, bo, gamma, beta):
    for b in (bq, bk, bv, bo):
        assert not np.any(np.asarray(b)), "nonzero bias unsupported"
    x = np.asarray(x, dtype=np.float32)
    mask = np.asarray(mask)
    n_un_all = (mask == 0).sum(axis=1)
    kc = min(max((int(n_un_all.max()) + 127) // 128, 2), ST)
    SC = kc * 128
    idxs = [np.argsort(mask[c], kind="stable")[:SC] for c in range(B)]
    key_idx = np.arange(SC).reshape(kc, 128).T  # [128, kc] global key index
    common = {
        "wq8": _pack_w(wq), "wk8": _pack_w(wk),
        "wv8": _pack_w(wv), "wo8": _pack_w(wo),
    }
    maps = []
    for c in range(B):
        xc = x[c][idxs[c]]
        live = (key_idx < int(n_un_all[c]))  # [128, kc]
        maps.append(dict(
            common,
            x32=np.ascontiguousarray(x[c]),
            xT8=_pack_xT(x[c]),
            xcT8=_pack_xT(xc),
            vmask=np.ascontiguousarray(live.astype(np.float32) * IWS),
            vones=np.ascontiguousarray(
                np.broadcast_to(live[:, :, None], (128, kc, H))
                .astype(ml_dtypes.float8_e4m3))))
    return maps, kc


def kernel(x, mask, wq, bq, wk, bk, wv, bv, wo, bo, gamma, beta):
    in_maps, kc = make_in_maps(x, mask, wq, bq, wk, bk, wv, bv, wo, bo,
                               gamma, beta)
    nc = _get_nc(1, kc)
    last_err = None
    for _ in range(3):
        try:
            res = run_bass_kernel_spmd(nc, in_maps, core_ids=list(range(B)))
            outv = np.stack([res.results[c]["out"] for c in range(B)], axis=0)
            gamma = np.asarray(gamma, np.float32)
            beta = np.asarray(beta, np.float32)
            if not (np.all(gamma == 1.0) and np.all(beta == 0.0)):
                outv = outv * gamma + beta
            return outv
        except Exception as e:  # transient NRT device errors: retry
            last_err = e
            time.sleep(5)
    raise last_err


# revision 16
# speedup vs baseline: 1.3309x; 1.0383x over previous
"""Multi-head attention block on 8 Trainium2 NeuronCores, data-parallel over
batch, fp8 (e4m3) matmul datapath with DoubleRow perf mode.

Shapes (hardcoded): B=8, S=1024, H=16, HD=64, D=1024. One batch element per
core. Host pre-transposes/casts x and the weights to fp8 (weights scaled by
64 for fp8 range; the 1/64 is folded into the movers), and compacts keys
(unmasked first). Masking is applied by ZEROING the v'/ones rows of masked
(tail) keys, so the exp needs no per-key bias and can run on either engine.

Per-core dataflow v2:
  phase 1: qT/kT (fp8) and v' ([128keys, kc, H, 72] + ones col at 64, tail
           rows zeroed) via DoubleRow fp8 matmuls; movers on ScalarE
           (Identity activation, scale=1/64 or the per-key v-mask).
  phase 2 per dt (2 heads), qh outer: per kt ONE row-tiled matmul PAIR
           computes both heads' scoresT[k,q] concurrently (head A in PE rows
           0-63, head B in rows 64-127, separate PSUM banks); E = exp(s/8)
           computed per (kt,qh) over [128, 2head*512] either on ScalarE
           (Exp) or on VectorE via a Schraudolph int8 bit-trick that emits
           fp8 bits directly; ctx' psum [65, 512] per head accumulates
           v'^T @ E DoubleRow over kt pairs (row 64 = denominator).
           Normalize: DVE reciprocal of the den row, GpSimd
           partition_broadcast, DVE mult+fp8-cast into ctxT.
  phase 3: out proj fp8 DoubleRow; t = x + proj/64 (DVE STT); LayerNorm
           stats via bn_stats/bn_aggr, rstd via batched DVE Newton rsqrt
           (no ScalarE table switch!); final apply on GpSimd.
"""
import sys
import time

sys.path.insert(0, "/opt/trn_rl_repo")

import numpy as np
import ml_dtypes

import concourse.bass as bass
import concourse.bacc as bacc
import concourse.tile as tile
from concourse import mybir
from concourse.bass_utils import run_bass_kernel_spmd

F32 = mybir.dt.float32
FP8 = mybir.dt.float8e4
INT32 = mybir.dt.int32
INT8 = mybir.dt.int8
AF = mybir.ActivationFunctionType
DR = mybir.MatmulPerfMode.DoubleRow
MUL = mybir.AluOpType.mult
ADD = mybir.AluOpType.add
RSH = mybir.AluOpType.logical_shift_right

B, S, H, HD = 8, 1024, 16, 64
D = H * HD
EPS = 1e-6
ST = S // 128    # 8 s-tiles
DT = D // 128    # 8 d-tiles
NH = S // 512    # 2 query halves
WS = 64.0        # weight scale for fp8
IWS = 1.0 / WS
VST = 72         # per-head stride in v' (64 v dims + ones col + pad)
SC_L = 11.5416 * 0.125   # Schraudolph exp: 8*log2(e) * score scale
SC_B = 55.654            # bias: 7*8 - 8*log2(1.0308) (round-to-nearest)
RSQRT_C = 0x5F3759DF     # Schraudolph rsqrt seed constant
RECIP_C = 0x7EF311C3     # Schraudolph reciprocal constant

# exp-engine split: (kt, qh) pairs routed to the DVE Schraudolph path;
# everything else uses ScalarE Exp. Tuned for ACT/DVE load balance.
D_SET = {(2, 0), (2, 1)}


def _emit_body(nc, tc, io, cst, kc, pools):
    (x32, out) = io
    (vmask_sb, qT, kT, vp, ctxT, xT_sb, xcT_sb,
     wq_sb, wk_sb, wv_sb, wo_sb, x_sb) = cst
    SC = kc * 128
    kpairs = kc // 2          # full DoubleRow key-tile pairs for AV
    ktail = kc - 2 * kpairs   # 0 or 1 leftover key tile

    # ---------------- phases 1+2 merged: projections feed heads ------
    # PSUM tags: "scp" [128,2,512]x3 = 6 banks (scores ring, also carries
    # projection chunks); "ctx" [65,512]x2 = 2 banks (ctx accumulators,
    # also phase-3 out-proj chunks). Pools live across reps so phase-3
    # tails overlap the next rep.
    if True:
        (Ep, nrm, ps, scps, epi, tke) = pools

        def vproj(st):
            # v' natural [keys, dcols], scattered to head slots; the
            # per-key scale (1/64, or 0 for masked tail keys) applies
            # the mask. Both 512-col halves go into one scp-ring tile,
            # drained by a single merged mover.
            vps = scps.tile([128, 2, 512], F32, tag="scp", name="vps")
            for et in range(2):
                for kp in range(0, DT, 2):
                    nc.tensor.matmul(
                        vps[:, et, :],
                        xcT_sb[:, kp:kp + 2, st * 128:(st + 1) * 128],
                        wv_sb[:, kp:kp + 2, et * 512:(et + 1) * 512],
                        start=(kp == 0), stop=(kp == DT - 2),
                        perf_mode=DR)
            nc.scalar.activation(
                vp[:, st, :, 0:64],
                vps.rearrange("p e (h j) -> p (e h) j", j=64),
                AF.Identity, scale=vmask_sb[:, st:st + 1])

        k_tiles = [(i * 512, min(512, SC - i * 512))
                   for i in range((SC + 511) // 512)]
        q_tiles = [(i * 512, 512) for i in range(NH)]

        def project(wsb_t, dstT, rhsT, mg, ntiles):
            qps = scps.tile([128, 2, 512], F32, tag="scp", name="qps")
            tot = 0
            for ci, (n0, nsz) in enumerate(ntiles):
                for kp in range(0, DT, 2):
                    nc.tensor.matmul(
                        qps[:, ci, 0:nsz],
                        wsb_t[:, kp:kp + 2, mg * 128:(mg + 1) * 128],
                        rhsT[:, kp:kp + 2, n0:n0 + nsz],
                        start=(kp == 0), stop=(kp == DT - 2),
                        perf_mode=DR)
                tot += nsz
            nc.scalar.activation(
                dstT[:, mg, 0:tot],
                qps.rearrange("p a b -> p (a b)")[:, 0:tot],
                AF.Identity, scale=IWS)

        def drain(c, hi, dt, q0):
            # normalize: Schraudolph recip of den row (DVE), partition
            # broadcast (GpSimd), multiply + fp8 cast into ctxT (DVE)
            base = hi * 64
            r0 = nrm.tile([1, 512], F32, tag=f"r0{hi}", name="r0")
            nc.vector.tensor_scalar(
                r0.bitcast(INT32), c[64:65, :].bitcast(INT32),
                -1, RECIP_C, op0=MUL, op1=ADD)
            rbc = nrm.tile([64, 512], F32, tag=f"rbc{hi}", name="rbc")
            nc.gpsimd.partition_broadcast(rbc, r0)
            nc.vector.tensor_mul(
                ctxT[base:base + 64, dt, q0:q0 + 512],
                c[0:64, :], rbc)

        def av_pair(cps2, qh, kt, hA, hB, e_pair):
            # DoubleRow AV over the key-tile pair ending at odd kt, or
            # the stride-0 pad trick for an odd-kc tail at even kt.
            last_pair = (kt % 2 == 1 and ktail == 0 and kt == kc - 1)
            if kt % 2 == 1:
                for hi, h in ((0, hA), (1, hB)):
                    nc.tensor.matmul(
                        cps2[hi],
                        vp[:, kt - 1:kt + 1, h, 0:65],
                        e_pair[:, 0:2, hi, :],
                        start=(kt == 1), stop=last_pair,
                        perf_mode=DR)
            else:
                for hi, h in ((0, hA), (1, hB)):
                    e0 = e_pair[:, 0, hi, :]
                    e00 = bass.AP(
                        tensor=e0.tensor, offset=e0.offset,
                        ap=[list(e0.ap[0]), [0, 2]]
                        + [list(a) for a in e0.ap[1:]])
                    nc.tensor.matmul(
                        cps2[hi],
                        vp[:, kt:kt + 2, h, 0:65],
                        e00,
                        start=(kc == 1), stop=True,
                        perf_mode=DR)

        def dummy_mm(c, rhs_view):
            # HAM keepalive: accumulate exact zeros (vp pad plane,
            # stride-0 pair) into a live ctx psum so the PE array never
            # idles long enough to re-throttle to K=4/8.
            z0 = vp[:, kc, 0, 0:65]
            z00 = bass.AP(
                tensor=z0.tensor, offset=z0.offset,
                ap=[list(z0.ap[0]), [0, 2]] + [list(a) for a in z0.ap[1:]])
            nc.tensor.matmul(c, z00, rhs_view, start=False, stop=False,
                             perf_mode=DR, skip_group_check=True)

        def chain(dt, qh):
            """Attention for heads (2dt, 2dt+1) over query half qh.
            AV pairs are emitted one kt late so the exp round-trip has a
            full extra kt of slack before the PE needs its result."""
            hA, hB = 2 * dt, 2 * dt + 1
            q0 = qh * 512
            cps = [ps.tile([65, 512], F32, tag="ctx", name="cps")
                   for _ in range(2)]
            e_pair = None
            pend = None
            for kt in range(kc):
                scp = scps.tile([128, 2, 512], F32, tag="scp", name="scp")
                # row-tiled concurrent pair: head A rows 0-63, head B
                # rows 64-127 of the PE array
                nc.tensor.matmul(
                    scp[:, 0, :],
                    kT[0:64, dt, kt * 128:(kt + 1) * 128],
                    qT[0:64, dt, q0:q0 + 512],
                    start=True, stop=True)
                nc.tensor.matmul(
                    scp[:, 1, :],
                    kT[64:128, dt, kt * 128:(kt + 1) * 128],
                    qT[64:128, dt, q0:q0 + 512],
                    start=True, stop=True)
                if kt % 2 == 0:
                    e_pair = Ep.tile([128, 2, 2, 512], FP8, tag=f"E{qh}",
                                     name="e_pair")
                if (kt, qh) in D_SET:
                    nc.vector.tensor_scalar(
                        e_pair[:, kt % 2, :, :].bitcast(INT8), scp,
                        SC_L, SC_B, op0=MUL, op1=ADD)
                else:
                    nc.scalar.activation(
                        e_pair[:, kt % 2, :, :], scp, AF.Exp, scale=0.125)
                if pend is not None:
                    av_pair(cps, qh, *pend)
                    pend = None
                if kt % 2 == 1 or kt == kc - 1:
                    pend = (kt, hA, hB, e_pair)
            av_pair(cps, qh, *pend)
            drain(cps[0], 0, dt, q0)
            drain(cps[1], 1, dt, q0)

        for st in range(kc):
            vproj(st)
        project(wq_sb, qT, xT_sb, 0, q_tiles)
        project(wk_sb, kT, xcT_sb, 0, k_tiles)
        for dt in range(DT):
            chain(dt, 0)
            if dt + 1 < DT:
                project(wq_sb, qT, xT_sb, dt + 1, q_tiles)
            chain(dt, 1)
            if dt + 1 < DT:
                project(wk_sb, kT, xcT_sb, dt + 1, k_tiles)

    # ---------------- phase 3: out proj + LayerNorm ----------------
    if True:
        t_tiles = []
        mvall = tke.tile([128, ST, 2], F32, tag="mva", name="mva")
        for qt in range(ST):
            x_t = x_sb[:, qt, :]
            t = tke.tile([128, D], F32, tag=f"tq{qt}", name="t")
            t_tiles.append(t)
            for et in range(2):
                pps = ps.tile([128, 512], F32, tag="ctx", name="pps")
                for dp in range(0, DT, 2):
                    nc.tensor.matmul(
                        pps,
                        ctxT[:, dp:dp + 2, qt * 128:(qt + 1) * 128],
                        wo_sb[:, dp:dp + 2, et * 512:(et + 1) * 512],
                        start=(dp == 0), stop=(dp == DT - 2),
                        perf_mode=DR)
                nc.vector.scalar_tensor_tensor(
                    out=t[:, et * 512:(et + 1) * 512],
                    in0=pps, scalar=IWS,
                    in1=x_t[:, et * 512:(et + 1) * 512],
                    op0=MUL, op1=ADD)
            stats = epi.tile([128, 2, nc.vector.BN_STATS_DIM], F32,
                             tag="stats", name="stats")
            tg = t.rearrange("p (g d) -> p g d", g=2)
            for g in range(2):
                nc.vector.bn_stats(stats[:, g, :], tg[:, g, :])
            nc.vector.bn_aggr(mvall[:, qt, :], stats)

        # batched Newton rsqrt on DVE: rstd = 1/sqrt(var + eps)
        vv = tke.tile([128, ST], F32, tag="nwv", name="nwv")
        nc.vector.tensor_scalar(vv, mvall[:, :, 1], EPS, None, op0=ADD)
        y = tke.tile([128, ST], F32, tag="nwy", name="nwy")
        nc.vector.tensor_scalar(y.bitcast(INT32), vv.bitcast(INT32),
                                1, None, op0=RSH)
        nc.vector.tensor_scalar(y.bitcast(INT32), y.bitcast(INT32),
                                -1, RSQRT_C, op0=MUL, op1=ADD)
        t1 = tke.tile([128, ST], F32, tag="nwt", name="nwt")
        for _ in range(2):  # two Newton iterations
            nc.vector.tensor_mul(t1, vv, y)
            nc.vector.tensor_mul(t1, t1, y)
            nc.vector.tensor_scalar(t1, t1, -0.5, 1.5, op0=MUL, op1=ADD)
            nc.vector.tensor_mul(y, y, t1)
        nmr = tke.tile([128, ST], F32, tag="nwm", name="nwm")
        nc.vector.scalar_tensor_tensor(
            out=nmr, in0=mvall[:, :, 0], scalar=-1.0, in1=y,
            op0=MUL, op1=MUL)

        for qt in range(ST):
            o_t = epi.tile([128, D], F32, tag="ot", name="o_t")
            nc.gpsimd.tensor_scalar(
                o_t, t_tiles[qt], y[:, qt:qt + 1], nmr[:, qt:qt + 1],
                op0=MUL, op1=ADD)
            nc.sync.dma_start(
                out=out[qt * 128:(qt + 1) * 128, :], in_=o_t)


def build_bass(reps=1, kc=8):
    nc = bacc.Bacc("TRN2", target_bir_lowering=False, debug=False)

    SC = kc * 128
    x32 = nc.dram_tensor("x32", [S, D], F32, kind="ExternalInput").ap()
    xT8 = nc.dram_tensor("xT8", [128, DT, S], FP8, kind="ExternalInput").ap()
    xcT8 = nc.dram_tensor("xcT8", [128, DT, SC], FP8,
                          kind="ExternalInput").ap()
    wq8 = nc.dram_tensor("wq8", [128, DT, D], FP8, kind="ExternalInput").ap()
    wk8 = nc.dram_tensor("wk8", [128, DT, D], FP8, kind="ExternalInput").ap()
    wv8 = nc.dram_tensor("wv8", [128, DT, D], FP8, kind="ExternalInput").ap()
    wo8 = nc.dram_tensor("wo8", [128, DT, D], FP8, kind="ExternalInput").ap()
    vmask = nc.dram_tensor("vmask", [128, kc], F32, kind="ExternalInput").ap()
    vones = nc.dram_tensor("vones", [128, kc, H], FP8,
                           kind="ExternalInput").ap()
    out = nc.dram_tensor("out", [S, D], F32, kind="ExternalOutput").ap()
    io = (x32, out)

    with tile.TileContext(nc) as tc:
        with tc.tile_pool(name="const", bufs=1) as const:
            vmask_sb = const.tile([128, kc], F32, name="vmask_sb")
            nc.sync.dma_start(out=vmask_sb, in_=vmask)
            # invocation-constant inputs: loaded once, resident in SBUF
            xT_sb = const.tile([128, DT, S], FP8, name="xT_sb")
            nc.sync.dma_start(out=xT_sb, in_=xT8)
            xcT_sb = const.tile([128, DT, SC], FP8, name="xcT_sb")
            nc.sync.dma_start(out=xcT_sb, in_=xcT8)
            wq_sb = const.tile([128, DT, D], FP8, name="wq_sb")
            nc.sync.dma_start(out=wq_sb, in_=wq8)
            wk_sb = const.tile([128, DT, D], FP8, name="wk_sb")
            nc.sync.dma_start(out=wk_sb, in_=wk8)
            wv_sb = const.tile([128, DT, D], FP8, name="wv_sb")
            nc.sync.dma_start(out=wv_sb, in_=wv8)
            wo_sb = const.tile([128, DT, D], FP8, name="wo_sb")
            nc.sync.dma_start(out=wo_sb, in_=wo8)
            x_sb = const.tile([128, ST, D], F32, name="x_sb")
            nc.sync.dma_start(out=x_sb,
                              in_=x32.rearrange("(t p) d -> p t d", p=128))
            qT = const.tile([128, DT, S], FP8, name="qT")
            kT = const.tile([128, DT, SC], FP8, name="kT")
            vp = const.tile([128, kc + 1, H, VST], FP8, name="vp")
            ctxT = const.tile([128, DT, S], FP8, name="ctxT")
            # ones column of v' (col 64 of each head slot): 1.0 for live
            # keys, 0.0 for masked tail keys (this applies the mask)
            nc.sync.dma_start(out=vp[:, 0:kc, :, 64], in_=vones)
            # zeroed pad key-plane: lets the odd-kc AV tail run DoubleRow
            # with a garbage E plane
            nc.vector.memset(vp[:, kc, :, :], 0.0)
            cst = (vmask_sb, qT, kT, vp, ctxT, xT_sb, xcT_sb,
                   wq_sb, wk_sb, wv_sb, wo_sb, x_sb)
            with (
                tc.tile_pool(name="Ep", bufs=4) as Ep,
                tc.tile_pool(name="nrm", bufs=2) as nrm,
                tc.tile_pool(name="ps", bufs=2, space="PSUM") as ps,
                tc.tile_pool(name="scp", bufs=3, space="PSUM") as scps,
                tc.tile_pool(name="epi", bufs=3) as epi,
                tc.tile_pool(name="tke", bufs=1) as tke,
            ):
                pools = (Ep, nrm, ps, scps, epi, tke)
                for _ in range(reps):
                    _emit_body(nc, tc, io, cst, kc, pools)

    nc.compile()
    return nc


_NC_CACHE = {}


def _get_nc(reps=1, kc=8):
    if (reps, kc) not in _NC_CACHE:
        _NC_CACHE[(reps, kc)] = build_bass(reps, kc)
    return _NC_CACHE[(reps, kc)]


def _pack_w(w):
    # [D, D] -> [128, DT, D] fp8 with w8[p, t, n] = w[t*128+p, n] * WS
    return np.ascontiguousarray(
        (np.asarray(w, np.float32) * WS).reshape(DT, 128, D)
        .transpose(1, 0, 2)).astype(ml_dtypes.float8_e4m3)


def _pack_xT(x):
    # [S', D] -> [128, DT, S'] fp8 with xT8[p, t, s] = x[s, t*128+p]
    return np.ascontiguousarray(
        np.asarray(x, np.float32).T.reshape(DT, 128, -1)
        .transpose(1, 0, 2)).astype(ml_dtypes.float8_e4m3)


def make_in_maps(x, mask, wq, bq, wk, bk, wv, bv, wo, bo, gamma, beta):
    for b in (bq, bk, bv, bo):
        assert not np.any(np.asarray(b)), "nonzero bias unsupported"
    x = np.asarray(x, dtype=np.float32)
    mask = np.asarray(mask)
    n_un_all = (mask == 0).sum(axis=1)
    kc = min(max((int(n_un_all.max()) + 127) // 128, 2), ST)
    SC = kc * 128
    idxs = [np.argsort(mask[c], kind="stable")[:SC] for c in range(B)]
    key_idx = np.arange(SC).reshape(kc, 128).T  # [128, kc] global key index
    common = {
        "wq8": _pack_w(wq), "wk8": _pack_w(wk),
        "wv8": _pack_w(wv), "wo8": _pack_w(wo),
    }
    maps = []
    for c in range(B):
        xc = x[c][idxs[c]]
        live = (key_idx < int(n_un_all[c]))  # [128, kc]
        maps.append(dict(
            common,
            x32=np.ascontiguousarray(x[c]),
            xT8=_pack_xT(x[c]),
            xcT8=_pack_xT(xc),
            vmask=np.ascontiguousarray(live.astype(np.float32) * IWS),
            vones=np.ascontiguousarray(
                np.broadcast_to(live[:, :, None], (128, kc, H))
                .astype(ml_dtypes.float8_e4m3))))
    return maps, kc


def kernel(x, mask, wq, bq, wk, bk, wv, bv, wo, bo, gamma, beta):
    in_maps, kc = make_in_maps(x, mask, wq, bq, wk, bk, wv, bv, wo, bo,
                               gamma, beta)
    nc = _get_nc(1, kc)
    last_err = None
    for _ in range(3):
        try:
            res = run_bass_kernel_spmd(nc, in_maps, core_ids=list(range(B)))
            outv = np.stack([res.results[c]["out"] for c in range(B)], axis=0)
            gamma = np.asarray(gamma, np.float32)
            beta = np.asarray(beta, np.float32)
            if not (np.all(gamma == 1.0) and np.all(beta == 0.0)):
                outv = outv * gamma + beta
            return outv
        except Exception as e:  # transient NRT device errors: retry
            last_err = e
            time.sleep(5)
    raise last_err
